# revision 3
# baseline (speedup 1.0000x reference)
"""Trainium2 Bass kernel for AttentiveSSMNoProjCyc (sparse_attention), v2.

Sharding: 8 cores = 2 batches x 4 head-groups (4 heads / 256 channels each).
Per core, [channel, time] layout, fp16 compute domain / bf16 exp domain:
  - SSM scans via one fused tensor_tensor_scan [128, 2*S] per d-tile
  - band scores use rope cancellation: diag = q.k, sub-diag = q.(R(-d)k)
    with a constant per-partition rotation (no full k-rope)
  - boundary keys: gather 33 cols of k, rope them at [128,48] cost
  - q-rope via PE permutation matmul + ACT copy + DVE/Pool fp16 TT
  - per-chunk softmax / combine pipeline
  - DMAs consolidated into packed blobs (HWDGE generation is serialized)
Host sums the 4 per-batch partials and transposes back.
"""
import numpy as np

import concourse.bass as bass
import concourse.mybir as mybir
from concourse.bass_utils import run_bass_kernel_spmd
from concourse.tile import TileContext
import concourse.tile as _tile_mod
from concourse.vector_clock import ScopedClock as _ScopedClock


def _split_drain_and_barrier(self, tick_clock, wait_clock):
    """Tail drain, with its sem waits spread over chained SP nops."""
    probe = self.nc.sync.nop()
    wait_clock.add_sem_waits(
        probe.ins, _ScopedClock({None: tick_clock.global_clock})
    )
    si = probe.ins.sync_info
    waits = list(si.on_wait) if si is not None else []
    upds = list(si.on_update) if si is not None else []
    MAXW = 1
    if len(waits) > MAXW:
        probe.ins.sync_info = mybir.SyncInfo(on_wait=waits[:MAXW],
                                             on_update=upds)
        for i in range(MAXW, len(waits), MAXW):
            extra = self.nc.sync.nop()
            extra.ins.sync_info = mybir.SyncInfo(
                on_wait=waits[i:i + MAXW], on_update=[])
    self.nc.sync.drain()

    self.nc.all_engine_barrier()
    assert self.sems is not None
    popped = self.nc._tile_sem_poison_stack.pop()
    assert popped is self._sem_poison
    self.nc.clear_and_free_semaphores(list(self.sems.allocated().values()))
    self.nc.all_engine_barrier()


_tile_mod.TileContext._drain_and_barrier = _split_drain_and_barrier


def _cap_sync_waits(nc, cap=1):
    """Hoist excess sync waits onto same-engine carrier NOPs."""
    nid = [0]

    def mknop(engine, waits):
        nid[0] += 1
        nop = mybir.InstNoOp(name=f"I-capw-{nid[0]}", ins=[], outs=[])
        nop.engine = engine
        nop.sync_info = mybir.SyncInfo(on_wait=list(waits), on_update=[])
        return nop

    for bb in nc.m.functions[0].blocks:
        il = bb.instructions
        i = 0
        while i < len(il):
            ins = il[i]
            si = ins.sync_info
            nw = len(si.on_wait) if si is not None else 0
            if nw > cap:
                waits = list(si.on_wait)
                ins.sync_info = mybir.SyncInfo(on_wait=waits[:cap],
                                               on_update=list(si.on_update))
                rest = waits[cap:]
                pos = i
                for j in range(0, len(rest), cap):
                    il.insert(pos, mknop(ins.engine, rest[j:j + cap]))
                    pos += 1
                    i += 1
            i += 1


B, S, D, H, HD = 2, 2048, 1024, 16, 64
NG = 4            # head-groups per batch
CH = 256          # channels per core (4 heads)
NB = 48           # padded boundary columns (33 real)
NBD = 112         # blockdiag boundary cols: head0 -> 0:48, head1 -> 64:112
NCHUNK = 4
CS = S // NCHUNK  # 512
F32 = mybir.dt.float32
F16 = mybir.dt.float16
BF16 = mybir.dt.bfloat16
AL = mybir.AluOpType
AF = mybir.ActivationFunctionType
NEG = -1e30

SHUF_XOR1 = [i ^ 1 for i in range(32)]

# packed fp16 const blob layout (columns)
CB_CBC = 0                 # [128, S]
CB_SBC = CB_CBC + S        # [128, S]
CB_CBCB = CB_SBC + S       # [128, NB]
CB_SBCB = CB_CBCB + NB     # [128, NB]
CB_OBLK = CB_SBCB + NB     # [128, 144]: 4x 36-col variants
CB_PERM = CB_OBLK + 144    # [128, 128]
CB_IDENT = CB_PERM + 128   # [128, 128]
CB_IND = CB_IDENT + 128    # [4, 256]
CB16_W = CB_IND + 256

# packed bf16 blob layout
BB_MASK = 0                # [NBD, S]
BB_OBV = BB_MASK + S       # [NBD, 8]
BB_INDB = BB_OBV + 8       # [4, 256]
BB_W = BB_INDB + 256

# packed fp32 blob: apar [128,4] | rotc [128,2]
C32_W = 6


def _boundaries():
    K_, LAYER_, NLAYERS_, MAXLEN_ = 64, 4, 16, 16384
    off = min(K_ - 1, LAYER_ * (K_ // NLAYERS_))
    bl = [b - off for b in range(K_ - 1, MAXLEN_, K_)]
    if bl[-1] != MAXLEN_ - 1:
        bl.append(MAXLEN_ - 1)
    if bl[0] != 0:
        bl.insert(0, 0)
    b = np.asarray(bl)
    b = b[b < S].copy()
    b[-1] = S - 1
    return b


BND = _boundaries()
NBR = len(BND)  # 33


def build_program(cap_waits=True):
    nc = bass.Bass()
    dp = nc.declare_dram_parameter
    xt = dp("xt", [D, S], F16, isOutput=False)
    wqt = dp("wqt", [128, 8 * CH], F16, isOutput=False)   # packed k-tiles
    wot = dp("wot", [128, 2 * D], F16, isOutput=False)    # packed k-tiles
    nrs = dp("nrs", [128, S], F16, isOutput=False)
    cb16 = dp("cb16", [128, CB16_W], F16, isOutput=False)
    cb32 = dp("cb32", [128, C32_W], F32, isOutput=False)
    cbbf = dp("cbbf", [128, BB_W], BF16, isOutput=False)
    outp = dp("outp", [D, S], F16, isOutput=True)

    with TileContext(nc) as tc, nc.allow_low_precision(
            reason="2e-2 output tolerance; fp16/bf16 validated vs reference"):
        with (
            tc.tile_pool(name="persist", bufs=1) as pp,
            tc.tile_pool(name="scanio", bufs=3) as sio,     # A4/B4/kv 8K tiles
            tc.tile_pool(name="chk", bufs=3) as ch,         # per-chunk f16 tiles
            tc.tile_pool(name="chsm", bufs=3) as cs,        # per-chunk small rows
            tc.tile_pool(name="small", bufs=2) as ck,
            tc.tile_pool(name="psQ", bufs=2, space="PSUM") as psQ,
            tc.tile_pool(name="psB", bufs=3, space="PSUM") as psB,
            tc.tile_pool(name="psT", bufs=1, space="PSUM") as psT,
            tc.tile_pool(name="psO", bufs=2, space="PSUM") as psO,
        ):
            # ============ DMAs (ordered: scan-critical first) ============
            x_t = [pp.tile([128, S], F16, tag=f"x{k}", name=f"x_t{k}")
                   for k in range(8)]
            for k in range(2):
                nc.sync.dma_start(out=x_t[k], in_=xt[k * 128:(k + 1) * 128, :])
            nrs_t = pp.tile([128, S], F16, tag="nrs")
            nc.sync.dma_start(out=nrs_t, in_=nrs[:, :])
            cb32_t = pp.tile([128, C32_W], F32, tag="cb32")
            nc.sync.dma_start(out=cb32_t, in_=cb32[:, :])
            cb16_t = pp.tile([128, CB16_W], F16, tag="cb16")
            nc.sync.dma_start(out=cb16_t, in_=cb16[:, :])
            wqt_t = pp.tile([128, 8 * CH], F16, tag="wqt")
            nc.sync.dma_start(out=wqt_t, in_=wqt[:, :])
            for k in range(2, 8):
                nc.sync.dma_start(out=x_t[k], in_=xt[k * 128:(k + 1) * 128, :])
            wot_t = pp.tile([128, 2 * D], F16, tag="wot")
            nc.sync.dma_start(out=wot_t, in_=wot[:, :])
            cbbf_t = pp.tile([128, BB_W], BF16, tag="cbbf")
            nc.sync.dma_start(out=cbbf_t, in_=cbbf[:, :])

            apar_t = cb32_t[:, 0:4]
            cosP = cb32_t[:, 4:5]
            sinPs = cb32_t[:, 5:6]
            cbc_t = cb16_t[:, CB_CBC:CB_CBC + S]
            sbc_t = cb16_t[:, CB_SBC:CB_SBC + S]
            cbcb_t = cb16_t[:, CB_CBCB:CB_CBCB + NB]
            sbcb_t = cb16_t[:, CB_SBCB:CB_SBCB + NB]
            oblk_t = cb16_t[:, CB_OBLK:CB_OBLK + 144]
            permw_t = cb16_t[:, CB_PERM:CB_PERM + 128]
            ident_t = cb16_t[:, CB_IDENT:CB_IDENT + 128]
            ind_t = cb16_t[0:4, CB_IND:CB_IND + 256]
            maskb_t = cbbf_t[0:NBD, BB_MASK:BB_MASK + S]
            obv_t = cbbf_t[0:NBD, BB_OBV:BB_OBV + 8]
            indb_t = cbbf_t[0:4, BB_INDB:BB_INDB + 256]

            asig = pp.tile([128, 4], F32, tag="asig")
            nc.scalar.activation(asig, apar_t, AF.Sigmoid)
            oma = pp.tile([128, 4], F32, tag="oma")  # 1 - sigmoid(a)
            nc.vector.tensor_scalar(out=oma, in0=asig, scalar1=-1.0,
                                    scalar2=1.0, op0=AL.mult, op1=AL.add)

            # ============ SSM scans (k and v fused per d-tile) ============
            kpre = [pp.tile([128, S], F16, tag=f"kp{dt}", name=f"kpre{dt}")
                    for dt in range(2)]
            v_t = [pp.tile([128, S], F16, tag=f"v{dt}", name=f"v_t{dt}")
                   for dt in range(2)]
            for dt in range(2):
                A4 = sio.tile([128, 2 * S], F16, tag="sc8k")
                B4 = sio.tile([128, 2 * S], F16, tag="sc8k")
                nc.vector.tensor_scalar(out=A4[:, 0:S], in0=nrs_t,
                                        scalar1=asig[:, dt:dt + 1],
                                        scalar2=None, op0=AL.mult)
                nc.vector.tensor_scalar(out=A4[:, S:2 * S], in0=nrs_t,
                                        scalar1=asig[:, 2 + dt:3 + dt],
                                        scalar2=None, op0=AL.mult)
                nc.vector.tensor_scalar(out=B4[:, 0:S], in0=x_t[dt],
                                        scalar1=oma[:, dt:dt + 1],
                                        scalar2=None, op0=AL.mult)
                nc.vector.tensor_scalar(out=B4[:, S:2 * S], in0=x_t[dt],
                                        scalar1=oma[:, 2 + dt:3 + dt],
                                        scalar2=None, op0=AL.mult)
                kv = sio.tile([128, 2 * S], F16, tag="sc8k")
                nc.vector.tensor_tensor_scan(out=kv, data0=A4, data1=B4,
                                             initial=0.0, op0=AL.mult,
                                             op1=AL.add)
                nc.vector.tensor_add(out=kpre[dt], in0=kv[:, 0:S],
                                     in1=x_t[dt])
                nc.vector.tensor_add(out=v_t[dt], in0=kv[:, S:2 * S],
                                     in1=x_t[dt])

            # ============ kd: constant rotation of kpre (for sub-diag) ======
            # kd = cosP*k + sinPs*shuffle(k); shuffle via PE permutation.
            kd = [pp.tile([128, S], F16, tag=f"kd{dt}", name=f"kd{dt}")
                  for dt in range(2)]
            for dt in range(2):
                t2k = sio.tile([128, S], F16, tag="t2k", bufs=1)
                for c in range(NCHUNK):
                    chs = slice(c * CS, (c + 1) * CS)
                    zp = psB.tile([128, CS], F32, tag="psb")
                    nc.tensor.matmul(zp, permw_t, kpre[dt][:, chs],
                                     start=True, stop=True)
                    nc.scalar.activation(t2k[:, chs], zp, AF.Copy,
                                         scale=sinPs)
                t1k = sio.tile([128, S], F16, tag="t1k", bufs=1)
                nc.vector.tensor_scalar(out=t1k, in0=kpre[dt], scalar1=cosP,
                                        scalar2=None, op0=AL.mult)
                nc.vector.tensor_add(out=kd[dt], in0=t1k, in1=t2k)

            # ============ boundary keys (gather 33 cols, rope there) =======
            kbd = [pp.tile([128, NBD], F16, tag=f"kbd{dt}", name=f"kbd{dt}")
                   for dt in range(2)]
            vbT = [pp.tile([128, 64], BF16, tag=f"vbT{dt}", name=f"vbT{dt}")
                   for dt in range(2)]
            for dt in range(2):
                kb = ck.tile([128, NB], F16, tag="kb")
                vb = ck.tile([128, NB], F16, tag="vb")
                for src_t, dst_t in ((kpre[dt], kb), (v_t[dt], vb)):
                    nc.vector.memset(dst_t[:, 33:NB], 0.0)
                    nc.vector.tensor_copy(out=dst_t[:, 0:1], in_=src_t[:, 0:1])
                    nc.vector.tensor_copy(
                        out=dst_t[:, 1:32],
                        in_=src_t.rearrange("p (a b) -> p a b", b=64)[:, 0:31, 47])
                    nc.vector.tensor_copy(out=dst_t[:, 32:33],
                                          in_=src_t[:, S - 1:S])
                # rope kb at boundary positions
                zb = ck.tile([128, NB], F16, tag="zb")
                nc.vector.stream_shuffle(zb, kb, SHUF_XOR1)
                t1b = ck.tile([128, NB], F16, tag="t1b")
                nc.vector.tensor_mul(out=t1b, in0=kb, in1=cbcb_t)
                t2b = ck.tile([128, NB], F16, tag="t2b")
                nc.vector.tensor_mul(out=t2b, in0=zb, in1=sbcb_t)
                krb = ck.tile([128, NB], F16, tag="krb")
                nc.vector.tensor_add(out=krb, in0=t1b, in1=t2b)
                # blockdiag [128, NBD]
                nc.vector.memset(kbd[dt], 0.0)
                nc.vector.tensor_copy(out=kbd[dt][0:64, 0:48], in_=krb[0:64, :])
                nc.vector.tensor_copy(out=kbd[dt][64:128, 64:112],
                                      in_=krb[64:128, :])
                # vbT: transpose vb blocks -> [48, 64] per head-half
                for hh in range(2):
                    tp = psT.tile([128, CS // 2], F16, tag="psbh",
                                  name=f"tp{dt}_{hh}")
                    nc.tensor.transpose(tp[0:48, 0:64],
                                        vb[hh * 64:(hh + 1) * 64, 0:48],
                                        ident_t[hh * 64:(hh + 1) * 64,
                                                hh * 64:(hh + 1) * 64],
                                        tile_position=(hh * 64, 0))
                    nc.scalar.activation(vbT[dt][hh * 64:hh * 64 + 48, :],
                                         tp[0:48, 0:64], AF.Copy)

            # ============ main per-chunk pipeline ============
            for c in range(NCHUNK):
                cl = c * CS
                chs = slice(cl, cl + CS)
                # --- Q projection ---
                accs = [psQ.tile([128, CS], F32, tag="mmacc",
                                 name=f"qacc{c}_{m}") for m in range(2)]
                for k in range(8):
                    for m in range(2):
                        nc.tensor.matmul(accs[m],
                                         wqt_t[:, k * CH + m * 128:
                                               k * CH + (m + 1) * 128],
                                         x_t[k][:, chs],
                                         start=(k == 0), stop=(k == 7))
                xq = [ch.tile([128, CS], F16, tag="xq", name=f"xq{c}_{dt}")
                      for dt in range(2)]
                for dt in range(2):
                    nc.scalar.activation(xq[dt], accs[dt], AF.Copy)
                # --- q rope (perm via PE; muls DVE+Pool) ---
                qr = [ch.tile([128, CS], F16, tag="qr", name=f"qr{c}_{dt}")
                      for dt in range(2)]
                for dt in range(2):
                    zq = psB.tile([128, CS], F32, tag="psb")
                    nc.tensor.matmul(zq, permw_t, xq[dt], start=True, stop=True)
                    zqs = ch.tile([128, CS], F16, tag="zqs")
                    nc.scalar.activation(zqs, zq, AF.Copy)
                    tq1 = ch.tile([128, CS], F16, tag="tq1")
                    nc.vector.tensor_mul(out=tq1, in0=xq[dt],
                                         in1=cbc_t[:, chs])
                    tq2 = ch.tile([128, CS], F16, tag="tq2")
                    nc.gpsimd.tensor_mul(out=tq2, in0=zqs, in1=sbc_t[:, chs])
                    nc.vector.tensor_add(out=qr[dt], in0=tq1, in1=tq2)

                # --- band scores (dt-accumulated, s1 rows 0:4 / s0 rows 4:8) --
                sp = psB.tile([128, CS], F32, tag="psb")
                prods = []
                for dt in range(2):
                    p1t = ch.tile([128, CS], F16, tag="prod", bufs=4)
                    nc.vector.tensor_mul(out=p1t, in0=xq[dt],
                                         in1=kpre[dt][:, chs])
                    p0t = ch.tile([128, CS], F16, tag="prod", bufs=4)
                    if c == 0:
                        nc.vector.memset(p0t[:, 0:1], 0.0)
                        nc.vector.tensor_mul(out=p0t[:, 1:CS],
                                             in0=xq[dt][:, 1:CS],
                                             in1=kd[dt][:, 0:CS - 1])
                    else:
                        nc.vector.tensor_mul(out=p0t, in0=xq[dt],
                                             in1=kd[dt][:, cl - 1:cl + CS - 1])
                    prods.append((p1t, p0t))
                for i, (dt, pi) in enumerate(((0, 0), (0, 1),
                                              (1, 0), (1, 1))):
                    nc.tensor.matmul(sp[0:36, :],
                                     oblk_t[:, 36 * i:36 * i + 36],
                                     prods[dt][pi], start=(i == 0),
                                     stop=(i == 3))
                if c == 0:
                    nc.vector.memset(sp[32:36, 0:1], NEG)
                e1 = cs.tile([4, CS], BF16, tag="erow", bufs=8, name=f"e1_{c}")
                nc.scalar.activation(e1, sp[0:4, :], AF.Exp, scale=0.125)
                e0 = cs.tile([4, CS], BF16, tag="erow", bufs=8, name=f"e0_{c}")
                nc.scalar.activation(e0, sp[32:36, :], AF.Exp, scale=0.125)

                # --- boundary scores ---
                embd = [ch.tile([NBD, CS], BF16, tag="embd", bufs=6,
                                name=f"embd{c}_{dt}") for dt in range(2)]
                for dt in range(2):
                    sb = psB.tile([128, CS], F32, tag="psb")
                    nc.tensor.matmul(sb[0:NBD, :], kbd[dt], qr[dt],
                                     start=True, stop=True)
                    eb = ch.tile([NBD, CS], BF16, tag="ebx")
                    nc.scalar.activation(eb, sb[0:NBD, :], AF.Exp, scale=0.125)
                    nc.gpsimd.tensor_mul(out=embd[dt], in0=eb,
                                         in1=maskb_t[:, chs])

                # --- denominator / p rows ---
                dbp = psB.tile([128, CS], F32, tag="psb")
                for dt in range(2):
                    nc.tensor.matmul(dbp[0:4, :], obv_t[:, 4 * dt:4 * dt + 4],
                                     embd[dt], start=(dt == 0), stop=(dt == 1))
                den = cs.tile([4, CS], BF16, tag="erow", bufs=8)
                nc.vector.tensor_add(out=den, in0=e1, in1=e0)
                den2 = cs.tile([4, CS], BF16, tag="erow", bufs=8)
                nc.vector.tensor_add(out=den2, in0=den, in1=dbp[0:4, :])
                rd = cs.tile([4, CS], BF16, tag="erow", bufs=8)
                nc.vector.reciprocal(rd, den2)
                p1 = cs.tile([4, CS], F16, tag="prow", bufs=4)
                p0 = cs.tile([4, CS], F16, tag="prow", bufs=4)
                nc.gpsimd.tensor_mul(out=p1, in0=e1, in1=rd)
                nc.gpsimd.tensor_mul(out=p0, in0=e0, in1=rd)

                # --- broadcasts + PV + combine ---
                attn = [ch.tile([128, CS], F16, tag="attn",
                                name=f"attn{c}_{dt}") for dt in range(2)]
                for dt in range(2):
                    io = dt * 128
                    p1b = psB.tile([128, CS], F32, tag="psb")
                    nc.tensor.matmul(p1b, ind_t[:, io:io + 128], p1,
                                     start=True, stop=True)
                    p0b = psB.tile([128, CS], F32, tag="psb")
                    nc.tensor.matmul(p0b, ind_t[:, io:io + 128], p0,
                                     start=True, stop=True)
                    rdb = psB.tile([128, CS], F32, tag="psb")
                    nc.tensor.matmul(rdb, indb_t[:, io:io + 128], rd,
                                     start=True, stop=True)
                    pv = psB.tile([128, CS], F32, tag="psb")
                    for hh in range(2):
                        nc.tensor.matmul(
                            pv[hh * 64:(hh + 1) * 64, :],
                            vbT[dt][hh * 64:hh * 64 + 48, :],
                            embd[dt][hh * 64:hh * 64 + 48, :],
                            start=True, stop=True,
                            tile_position=(hh * 64, hh * 64))
                    # SBUF staging: p1b/p0b/pvs on ACT; m3 takes rdb from PSUM
                    p1s = ch.tile([128, CS], F16, tag="pbs", bufs=7)
                    nc.scalar.activation(p1s, p1b, AF.Copy)
                    p0s = ch.tile([128, CS], F16, tag="pbs", bufs=7)
                    nc.scalar.activation(p0s, p0b, AF.Copy)
                    pvs = ch.tile([128, CS], BF16, tag="pbs", bufs=7)
                    nc.scalar.activation(pvs, pv, AF.Copy)
                    m1 = ch.tile([128, CS], F16, tag="mt", bufs=8)
                    nc.vector.tensor_mul(out=m1, in0=v_t[dt][:, chs], in1=p1s)
                    m2 = ch.tile([128, CS], F16, tag="mt", bufs=8)
                    if c == 0:
                        nc.vector.memset(m2[:, 0:1], 0.0)
                        nc.vector.tensor_mul(out=m2[:, 1:CS],
                                             in0=v_t[dt][:, 0:CS - 1],
                                             in1=p0s[:, 1:CS])
                    else:
                        nc.vector.tensor_mul(out=m2,
                                             in0=v_t[dt][:, cl - 1:cl + CS - 1],
                                             in1=p0s)
                    m3 = ch.tile([128, CS], F16, tag="mt", bufs=8)
                    nc.vector.tensor_mul(out=m3, in0=pvs, in1=rdb)
                    m4 = ch.tile([128, CS], F16, tag="mt", bufs=8)
                    nc.vector.tensor_add(out=m4, in0=m1, in1=m2)
                    nc.vector.tensor_add(out=attn[dt], in0=m4, in1=m3)

                # --- output projection ---
                for m in range(8):
                    acc = psO.tile([128, CS], F32, tag="oacc",
                                   name=f"oacc{c}_{m}")
                    for k in range(2):
                        nc.tensor.matmul(acc,
                                         wot_t[:, k * D + m * 128:
                                               k * D + (m + 1) * 128],
                                         attn[k], start=(k == 0), stop=(k == 1))
                    stage = ch.tile([128, CS], F16, tag="ostage", bufs=6,
                                    name=f"ost{c}_{m}")
                    if m % 2 == 0:
                        nc.scalar.activation(stage, acc, AF.Copy)
                    else:
                        nc.vector.tensor_copy(out=stage, in_=acc)
                    nc.scalar.dma_start(out=outp[m * 128:(m + 1) * 128, chs],
                                        in_=stage)
    if cap_waits:
        _cap_sync_waits(nc)
    return nc


# ---------------- host side ----------------

def _host_consts(fc):
    C = np.zeros((128, S), np.float32)
    Sg = np.zeros((128, S), np.float32)
    for p in range(128):
        i = (p % 64) // 2
        if p % 2 == 0:
            C[p] = fc[:, i, 0, 0]
            Sg[p] = fc[:, i, 0, 1]
        else:
            C[p] = fc[:, i, 1, 1]
            Sg[p] = fc[:, i, 1, 0]
    starts = np.concatenate([[0], BND[:-1] + 1])
    nrs1 = np.ones(S, np.float32)
    nrs1[starts] = 0.0
    nrs = np.broadcast_to(nrs1, (128, S)).astype(np.float16).copy()
    # constant one-step rotation R(-delta): kd[2i] = cos k[2i] + sin k[2i+1]
    inv = (1.0 / (10000.0 ** (np.arange(0, HD, 2, dtype=np.float64) / HD)))
    rotc = np.zeros((128, 2), np.float32)
    for p in range(128):
        i = (p % 64) // 2
        rotc[p, 0] = np.cos(inv[i])
        rotc[p, 1] = np.sin(inv[i]) * (1.0 if p % 2 == 0 else -1.0)
    # fp16 const blob
    cb16 = np.zeros((128, CB16_W), np.float32)
    cb16[:, CB_CBC:CB_CBC + S] = C
    cb16[:, CB_SBC:CB_SBC + S] = Sg
    cb16[:, CB_CBCB + 0:CB_CBCB + NBR] = C[:, BND]
    cb16[:, CB_SBCB + 0:CB_SBCB + NBR] = Sg[:, BND]
    for i, (dt, pi) in enumerate(((0, 0), (0, 1), (1, 0), (1, 1))):
        base = CB_OBLK + 36 * i + 32 * pi + 2 * dt
        cb16[0:64, base + 0] = 1.0
        cb16[64:128, base + 1] = 1.0
    for p in range(128):
        cb16[p, CB_PERM + (p ^ 1)] = 1.0
        cb16[p, CB_IDENT + p] = 1.0
    ind = np.zeros((4, 256), np.float32)
    ind[0, 0:64] = 1.0
    ind[1, 64:128] = 1.0
    ind[2, 128:192] = 1.0
    ind[3, 192:256] = 1.0
    cb16[0:4, CB_IND:CB_IND + 256] = ind
    # bf16 blob
    cbbf = np.zeros((128, BB_W), np.float32)
    t = np.arange(S)
    for hh in range(2):
        for jb in range(NBR):
            cbbf[hh * 64 + jb, BB_MASK:BB_MASK + S] = (
                t >= BND[jb] + 2).astype(np.float32)
    cbbf[0:48, BB_OBV + 0] = 1.0
    cbbf[64:112, BB_OBV + 1] = 1.0
    cbbf[0:48, BB_OBV + 6] = 1.0
    cbbf[64:112, BB_OBV + 7] = 1.0
    cbbf[0:4, BB_INDB:BB_INDB + 256] = ind
    return nrs, rotc, cb16, cbbf


_prog = None


def make_in_maps(x, fc, wq_, wo_, a_k_, a_v_):
    nrs, rotc, cb16, cbbf = _host_consts(fc)
    import ml_dtypes
    bf = ml_dtypes.bfloat16
    cb16_h = cb16.astype(np.float16)
    cbbf_h = cbbf.astype(bf)
    in_maps, metas = [], []
    for b in range(B):
        xT = np.ascontiguousarray(x[b].T)
        for g in range(NG):
            c0 = g * CH
            perm = np.concatenate([np.arange(c0, c0 + CH),
                                   np.arange(0, c0),
                                   np.arange(c0 + CH, D)]).astype(np.int64)
            xt_core = np.ascontiguousarray(xT[perm]).astype(np.float16)
            wq_core = wq_[c0:c0 + CH, :].T[perm]          # [D, CH]
            wqt_pack = np.ascontiguousarray(
                wq_core.reshape(8, 128, CH).transpose(1, 0, 2).reshape(
                    128, 8 * CH)).astype(np.float16)
            wo_core = wo_[:, c0:c0 + CH].T                # [CH, D]
            wot_pack = np.ascontiguousarray(
                wo_core.reshape(2, 128, D).transpose(1, 0, 2).reshape(
                    128, 2 * D)).astype(np.float16)
            apar = np.stack([a_k_[c0:c0 + 128], a_k_[c0 + 128:c0 + 256],
                             a_v_[c0:c0 + 128], a_v_[c0 + 128:c0 + 256]],
                            axis=1).astype(np.float32)
            cb32_h = np.concatenate([apar, rotc], axis=1).astype(np.float32)
            in_maps.append({
                "xt": xt_core, "wqt": wqt_pack, "wot": wot_pack,
                "nrs": nrs, "cb16": cb16_h, "cb32": cb32_h, "cbbf": cbbf_h,
            })
            metas.append((b, g))
    return in_maps, metas


def kernel(x, freq_cis, wq, wo, a_k, a_v):
    global _prog
    x = np.asarray(x, np.float32)
    fc = np.asarray(freq_cis, np.float32)
    wq_ = np.asarray(wq, np.float32)
    wo_ = np.asarray(wo, np.float32)
    a_k_ = np.asarray(a_k, np.float32)
    a_v_ = np.asarray(a_v, np.float32)
    in_maps, metas = make_in_maps(x, fc, wq_, wo_, a_k_, a_v_)
    if _prog is None:
        _prog = build_program()
    res = run_bass_kernel_spmd(_prog, in_maps, core_ids=list(range(8)))
    out = np.zeros((B, S, D), np.float32)
    for (b, g), r in zip(metas, res.results):
        out[b] += np.asarray(r["outp"], np.float32).T
    return out


if __name__ == "__main__":
    build_program()
    print("program built ok")


# revision 4
# speedup vs baseline: 1.0074x; 1.0074x over previous
"""Trainium2 Bass kernel for AttentiveSSMNoProjCyc (sparse_attention), v2.

Sharding: 8 cores = 2 batches x 4 head-groups (4 heads / 256 channels each).
Per core, [channel, time] layout, fp16 compute domain / bf16 exp domain:
  - SSM scans via one fused tensor_tensor_scan [128, 2*S] per d-tile
  - band scores use rope cancellation: diag = q.k, sub-diag = q.(R(-d)k)
    with a constant per-partition rotation (no full k-rope)
  - boundary keys: gather 33 cols of k, rope them at [128,48] cost
  - q-rope via PE permutation matmul + ACT copy + DVE/Pool fp16 TT
  - per-chunk softmax / combine pipeline
  - DMAs consolidated into packed blobs (HWDGE generation is serialized)
Host sums the 4 per-batch partials and transposes back.
"""
import numpy as np

import concourse.bass as bass
import concourse.mybir as mybir
from concourse.bass_utils import run_bass_kernel_spmd
from concourse.tile import TileContext
import concourse.tile as _tile_mod
from concourse.vector_clock import ScopedClock as _ScopedClock


def _split_drain_and_barrier(self, tick_clock, wait_clock):
    """Tail drain, with its sem waits spread over chained SP nops."""
    probe = self.nc.sync.nop()
    wait_clock.add_sem_waits(
        probe.ins, _ScopedClock({None: tick_clock.global_clock})
    )
    si = probe.ins.sync_info
    waits = list(si.on_wait) if si is not None else []
    upds = list(si.on_update) if si is not None else []
    MAXW = 1
    if len(waits) > MAXW:
        probe.ins.sync_info = mybir.SyncInfo(on_wait=waits[:MAXW],
                                             on_update=upds)
        for i in range(MAXW, len(waits), MAXW):
            extra = self.nc.sync.nop()
            extra.ins.sync_info = mybir.SyncInfo(
                on_wait=waits[i:i + MAXW], on_update=[])
    self.nc.sync.drain()

    self.nc.all_engine_barrier()
    assert self.sems is not None
    popped = self.nc._tile_sem_poison_stack.pop()
    assert popped is self._sem_poison
    self.nc.clear_and_free_semaphores(list(self.sems.allocated().values()))
    self.nc.all_engine_barrier()


_tile_mod.TileContext._drain_and_barrier = _split_drain_and_barrier


def _cap_sync_waits(nc, cap=1):
    """Hoist excess sync waits onto same-engine carrier NOPs."""
    nid = [0]

    def mknop(engine, waits):
        nid[0] += 1
        nop = mybir.InstNoOp(name=f"I-capw-{nid[0]}", ins=[], outs=[])
        nop.engine = engine
        nop.sync_info = mybir.SyncInfo(on_wait=list(waits), on_update=[])
        return nop

    for bb in nc.m.functions[0].blocks:
        il = bb.instructions
        i = 0
        while i < len(il):
            ins = il[i]
            si = ins.sync_info
            nw = len(si.on_wait) if si is not None else 0
            if nw > cap:
                waits = list(si.on_wait)
                ins.sync_info = mybir.SyncInfo(on_wait=waits[:cap],
                                               on_update=list(si.on_update))
                rest = waits[cap:]
                pos = i
                for j in range(0, len(rest), cap):
                    il.insert(pos, mknop(ins.engine, rest[j:j + cap]))
                    pos += 1
                    i += 1
            i += 1


B, S, D, H, HD = 2, 2048, 1024, 16, 64
NG = 4            # head-groups per batch
CH = 256          # channels per core (4 heads)
NB = 48           # padded boundary columns (33 real)
NBD = 112         # blockdiag boundary cols: head0 -> 0:48, head1 -> 64:112
NCHUNK = 4
CS = S // NCHUNK  # 512
F32 = mybir.dt.float32
F16 = mybir.dt.float16
BF16 = mybir.dt.bfloat16
AL = mybir.AluOpType
AF = mybir.ActivationFunctionType
NEG = -1e30

SHUF_XOR1 = [i ^ 1 for i in range(32)]

# packed fp16 const blob layout (columns)
CB_CBC = 0                 # [128, S]
CB_SBC = CB_CBC + S        # [128, S]
CB_CBCB = CB_SBC + S       # [128, NB]
CB_SBCB = CB_CBCB + NB     # [128, NB]
CB_OBLK = CB_SBCB + NB     # [128, 144]: 4x 36-col variants
CB_PERM = CB_OBLK + 144    # [128, 128]
CB_IDENT = CB_PERM + 128   # [128, 128]
CB_IND = CB_IDENT + 128    # [4, 256]
CB16_W = CB_IND + 256

# packed bf16 blob layout
BB_MASK = 0                # [NBD, S]
BB_OBV = BB_MASK + S       # [NBD, 8]
BB_INDB = BB_OBV + 8       # [4, 256]
BB_W = BB_INDB + 256

# packed fp32 blob: apar [128,4] | rotc [128,2]
C32_W = 6


def _boundaries():
    K_, LAYER_, NLAYERS_, MAXLEN_ = 64, 4, 16, 16384
    off = min(K_ - 1, LAYER_ * (K_ // NLAYERS_))
    bl = [b - off for b in range(K_ - 1, MAXLEN_, K_)]
    if bl[-1] != MAXLEN_ - 1:
        bl.append(MAXLEN_ - 1)
    if bl[0] != 0:
        bl.insert(0, 0)
    b = np.asarray(bl)
    b = b[b < S].copy()
    b[-1] = S - 1
    return b


BND = _boundaries()
NBR = len(BND)  # 33


def build_program(cap_waits=True):
    nc = bass.Bass()
    dp = nc.declare_dram_parameter
    xt = dp("xt", [D, S], F16, isOutput=False)
    wqt = dp("wqt", [128, 8 * CH], F16, isOutput=False)   # packed k-tiles
    wot = dp("wot", [128, 2 * D], F16, isOutput=False)    # packed k-tiles
    nrs = dp("nrs", [128, S], F16, isOutput=False)
    cb16 = dp("cb16", [128, CB16_W], F16, isOutput=False)
    cb32 = dp("cb32", [128, C32_W], F32, isOutput=False)
    cbbf = dp("cbbf", [128, BB_W], BF16, isOutput=False)
    outp = dp("outp", [D, S], F16, isOutput=True)

    with TileContext(nc) as tc, nc.allow_low_precision(
            reason="2e-2 output tolerance; fp16/bf16 validated vs reference"):
        with (
            tc.tile_pool(name="persist", bufs=1) as pp,
            tc.tile_pool(name="scanio", bufs=3) as sio,     # A4/B4/kv 8K tiles
            tc.tile_pool(name="chk", bufs=3) as ch,         # per-chunk f16 tiles
            tc.tile_pool(name="chsm", bufs=3) as cs,        # per-chunk small rows
            tc.tile_pool(name="small", bufs=2) as ck,
            tc.tile_pool(name="psQ", bufs=2, space="PSUM") as psQ,
            tc.tile_pool(name="psB", bufs=3, space="PSUM") as psB,
            tc.tile_pool(name="psT", bufs=1, space="PSUM") as psT,
            tc.tile_pool(name="psO", bufs=2, space="PSUM") as psO,
        ):
            # ============ DMAs (ordered: scan-critical first) ============
            x_t = [pp.tile([128, S], F16, tag=f"x{k}", name=f"x_t{k}")
                   for k in range(8)]
            for k in range(2):
                nc.sync.dma_start(out=x_t[k], in_=xt[k * 128:(k + 1) * 128, :])
            nrs_t = pp.tile([128, S], F16, tag="nrs")
            nc.sync.dma_start(out=nrs_t, in_=nrs[:, :])
            cb32_t = pp.tile([128, C32_W], F32, tag="cb32")
            nc.sync.dma_start(out=cb32_t, in_=cb32[:, :])
            cb16_t = pp.tile([128, CB16_W], F16, tag="cb16")
            nc.sync.dma_start(out=cb16_t, in_=cb16[:, :])
            wqt_t = pp.tile([128, 8 * CH], F16, tag="wqt")
            nc.sync.dma_start(out=wqt_t, in_=wqt[:, :])
            for k in range(2, 8):
                nc.sync.dma_start(out=x_t[k], in_=xt[k * 128:(k + 1) * 128, :])
            wot_t = pp.tile([128, 2 * D], F16, tag="wot")
            nc.sync.dma_start(out=wot_t, in_=wot[:, :])
            cbbf_t = pp.tile([128, BB_W], BF16, tag="cbbf")
            nc.sync.dma_start(out=cbbf_t, in_=cbbf[:, :])

            apar_t = cb32_t[:, 0:4]
            cosP = cb32_t[:, 4:5]
            sinPs = cb32_t[:, 5:6]
            cbc_t = cb16_t[:, CB_CBC:CB_CBC + S]
            sbc_t = cb16_t[:, CB_SBC:CB_SBC + S]
            cbcb_t = cb16_t[:, CB_CBCB:CB_CBCB + NB]
            sbcb_t = cb16_t[:, CB_SBCB:CB_SBCB + NB]
            oblk_t = cb16_t[:, CB_OBLK:CB_OBLK + 144]
            permw_t = cb16_t[:, CB_PERM:CB_PERM + 128]
            ident_t = cb16_t[:, CB_IDENT:CB_IDENT + 128]
            ind_t = cb16_t[0:4, CB_IND:CB_IND + 256]
            maskb_t = cbbf_t[0:NBD, BB_MASK:BB_MASK + S]
            obv_t = cbbf_t[0:NBD, BB_OBV:BB_OBV + 8]
            indb_t = cbbf_t[0:4, BB_INDB:BB_INDB + 256]

            asig = pp.tile([128, 4], F32, tag="asig")
            nc.scalar.activation(asig, apar_t, AF.Sigmoid)
            oma = pp.tile([128, 4], F32, tag="oma")  # 1 - sigmoid(a)
            nc.vector.tensor_scalar(out=oma, in0=asig, scalar1=-1.0,
                                    scalar2=1.0, op0=AL.mult, op1=AL.add)

            # ============ SSM scans (k and v fused per d-tile) ============
            kpre = [pp.tile([128, S], F16, tag=f"kp{dt}", name=f"kpre{dt}")
                    for dt in range(2)]
            v_t = [pp.tile([128, S], F16, tag=f"v{dt}", name=f"v_t{dt}")
                   for dt in range(2)]
            for dt in range(2):
                A4 = sio.tile([128, 2 * S], F16, tag="sc8k")
                B4 = sio.tile([128, 2 * S], F16, tag="sc8k")
                nc.vector.tensor_scalar(out=A4[:, 0:S], in0=nrs_t,
                                        scalar1=asig[:, dt:dt + 1],
                                        scalar2=None, op0=AL.mult)
                nc.vector.tensor_scalar(out=A4[:, S:2 * S], in0=nrs_t,
                                        scalar1=asig[:, 2 + dt:3 + dt],
                                        scalar2=None, op0=AL.mult)
                nc.vector.tensor_scalar(out=B4[:, 0:S], in0=x_t[dt],
                                        scalar1=oma[:, dt:dt + 1],
                                        scalar2=None, op0=AL.mult)
                nc.vector.tensor_scalar(out=B4[:, S:2 * S], in0=x_t[dt],
                                        scalar1=oma[:, 2 + dt:3 + dt],
                                        scalar2=None, op0=AL.mult)
                kv = sio.tile([128, 2 * S], F16, tag="sc8k")
                nc.vector.tensor_tensor_scan(out=kv, data0=A4, data1=B4,
                                             initial=0.0, op0=AL.mult,
                                             op1=AL.add)
                nc.vector.tensor_add(out=kpre[dt], in0=kv[:, 0:S],
                                     in1=x_t[dt])
                nc.vector.tensor_add(out=v_t[dt], in0=kv[:, S:2 * S],
                                     in1=x_t[dt])

            # ============ kd: constant rotation of kpre (for sub-diag) ======
            # kd = cosP*k + sinPs*shuffle(k); shuffle via PE permutation.
            kd = [pp.tile([128, S], F16, tag=f"kd{dt}", name=f"kd{dt}")
                  for dt in range(2)]
            for dt in range(2):
                t2k = sio.tile([128, S], F16, tag="t2k", bufs=1)
                for c in range(NCHUNK):
                    chs = slice(c * CS, (c + 1) * CS)
                    zp = psB.tile([128, CS], F32, tag="psb")
                    nc.tensor.matmul(zp, permw_t, kpre[dt][:, chs],
                                     start=True, stop=True)
                    nc.scalar.activation(t2k[:, chs], zp, AF.Copy,
                                         scale=sinPs)
                t1k = sio.tile([128, S], F16, tag="t1k", bufs=1)
                nc.vector.tensor_scalar(out=t1k, in0=kpre[dt], scalar1=cosP,
                                        scalar2=None, op0=AL.mult)
                nc.vector.tensor_add(out=kd[dt], in0=t1k, in1=t2k)

            # ============ boundary keys (gather 33 cols, rope there) =======
            kbd = [pp.tile([128, NBD], F16, tag=f"kbd{dt}", name=f"kbd{dt}")
                   for dt in range(2)]
            vbT = [pp.tile([128, 64], BF16, tag=f"vbT{dt}", name=f"vbT{dt}")
                   for dt in range(2)]
            for dt in range(2):
                kb = ck.tile([128, NB], F16, tag="kb")
                vb = ck.tile([128, NB], F16, tag="vb")
                for src_t, dst_t in ((kpre[dt], kb), (v_t[dt], vb)):
                    nc.vector.memset(dst_t[:, 33:NB], 0.0)
                    nc.vector.tensor_copy(out=dst_t[:, 0:1], in_=src_t[:, 0:1])
                    nc.vector.tensor_copy(
                        out=dst_t[:, 1:32],
                        in_=src_t.rearrange("p (a b) -> p a b", b=64)[:, 0:31, 47])
                    nc.vector.tensor_copy(out=dst_t[:, 32:33],
                                          in_=src_t[:, S - 1:S])
                # rope kb at boundary positions
                zb = ck.tile([128, NB], F16, tag="zb")
                nc.vector.stream_shuffle(zb, kb, SHUF_XOR1)
                t1b = ck.tile([128, NB], F16, tag="t1b")
                nc.vector.tensor_mul(out=t1b, in0=kb, in1=cbcb_t)
                t2b = ck.tile([128, NB], F16, tag="t2b")
                nc.vector.tensor_mul(out=t2b, in0=zb, in1=sbcb_t)
                krb = ck.tile([128, NB], F16, tag="krb")
                nc.vector.tensor_add(out=krb, in0=t1b, in1=t2b)
                # blockdiag [128, NBD]
                nc.vector.memset(kbd[dt], 0.0)
                nc.vector.tensor_copy(out=kbd[dt][0:64, 0:48], in_=krb[0:64, :])
                nc.vector.tensor_copy(out=kbd[dt][64:128, 64:112],
                                      in_=krb[64:128, :])
                # vbT: transpose vb blocks -> [48, 64] per head-half
                for hh in range(2):
                    tp = psT.tile([128, CS // 2], F16, tag="psbh",
                                  name=f"tp{dt}_{hh}")
                    nc.tensor.transpose(tp[0:48, 0:64],
                                        vb[hh * 64:(hh + 1) * 64, 0:48],
                                        ident_t[hh * 64:(hh + 1) * 64,
                                                hh * 64:(hh + 1) * 64],
                                        tile_position=(hh * 64, 0))
                    nc.scalar.activation(vbT[dt][hh * 64:hh * 64 + 48, :],
                                         tp[0:48, 0:64], AF.Copy)

            # ============ main per-chunk pipeline ============
            for c in range(NCHUNK):
                cl = c * CS
                chs = slice(cl, cl + CS)
                # --- Q projection ---
                accs = [psQ.tile([128, CS], F32, tag="mmacc",
                                 name=f"qacc{c}_{m}") for m in range(2)]
                for k in range(8):
                    for m in range(2):
                        nc.tensor.matmul(accs[m],
                                         wqt_t[:, k * CH + m * 128:
                                               k * CH + (m + 1) * 128],
                                         x_t[k][:, chs],
                                         start=(k == 0), stop=(k == 7))
                xq = [ch.tile([128, CS], F16, tag="xq", name=f"xq{c}_{dt}")
                      for dt in range(2)]
                for dt in range(2):
                    nc.scalar.activation(xq[dt], accs[dt], AF.Copy)
                # --- q rope (perm via PE; muls DVE+Pool) ---
                qr = [ch.tile([128, CS], F16, tag="qr", name=f"qr{c}_{dt}")
                      for dt in range(2)]
                for dt in range(2):
                    zq = psB.tile([128, CS], F32, tag="psb")
                    nc.tensor.matmul(zq, permw_t, xq[dt], start=True, stop=True)
                    zqs = ch.tile([128, CS], F16, tag="zqs")
                    nc.scalar.activation(zqs, zq, AF.Copy)
                    tq1 = ch.tile([128, CS], F16, tag="tq1")
                    nc.vector.tensor_mul(out=tq1, in0=xq[dt],
                                         in1=cbc_t[:, chs])
                    tq2 = ch.tile([128, CS], F16, tag="tq2")
                    nc.gpsimd.tensor_mul(out=tq2, in0=zqs, in1=sbc_t[:, chs])
                    nc.vector.tensor_add(out=qr[dt], in0=tq1, in1=tq2)

                # --- band scores (dt-accumulated, s1 rows 0:4 / s0 rows 4:8) --
                sp = psB.tile([128, CS], F32, tag="psb")
                prods = []
                for dt in range(2):
                    p1t = ch.tile([128, CS], F16, tag="prod", bufs=4)
                    nc.vector.tensor_mul(out=p1t, in0=xq[dt],
                                         in1=kpre[dt][:, chs])
                    p0t = ch.tile([128, CS], F16, tag="prod", bufs=4)
                    if c == 0:
                        nc.vector.memset(p0t[:, 0:1], 0.0)
                        nc.vector.tensor_mul(out=p0t[:, 1:CS],
                                             in0=xq[dt][:, 1:CS],
                                             in1=kd[dt][:, 0:CS - 1])
                    else:
                        nc.vector.tensor_mul(out=p0t, in0=xq[dt],
                                             in1=kd[dt][:, cl - 1:cl + CS - 1])
                    prods.append((p1t, p0t))
                for i, (dt, pi) in enumerate(((0, 0), (0, 1),
                                              (1, 0), (1, 1))):
                    nc.tensor.matmul(sp[0:36, :],
                                     oblk_t[:, 36 * i:36 * i + 36],
                                     prods[dt][pi], start=(i == 0),
                                     stop=(i == 3))
                if c == 0:
                    nc.vector.memset(sp[32:36, 0:1], NEG)
                e1 = cs.tile([4, CS], BF16, tag="erow", bufs=8, name=f"e1_{c}")
                nc.scalar.activation(e1, sp[0:4, :], AF.Exp, scale=0.125)
                e0 = cs.tile([4, CS], BF16, tag="erow", bufs=8, name=f"e0_{c}")
                nc.scalar.activation(e0, sp[32:36, :], AF.Exp, scale=0.125)

                # --- boundary scores ---
                embd = [ch.tile([NBD, CS], BF16, tag="embd", bufs=6,
                                name=f"embd{c}_{dt}") for dt in range(2)]
                for dt in range(2):
                    sb = psB.tile([128, CS], F32, tag="psb")
                    nc.tensor.matmul(sb[0:NBD, :], kbd[dt], qr[dt],
                                     start=True, stop=True)
                    eb = ch.tile([NBD, CS], BF16, tag="ebx")
                    nc.scalar.activation(eb, sb[0:NBD, :], AF.Exp, scale=0.125)
                    nc.gpsimd.tensor_mul(out=embd[dt], in0=eb,
                                         in1=maskb_t[:, chs])

                # --- denominator / p rows ---
                dbp = psB.tile([128, CS], F32, tag="psb")
                for dt in range(2):
                    nc.tensor.matmul(dbp[0:4, :], obv_t[:, 4 * dt:4 * dt + 4],
                                     embd[dt], start=(dt == 0), stop=(dt == 1))
                den = cs.tile([4, CS], BF16, tag="erow", bufs=8)
                nc.vector.tensor_add(out=den, in0=e1, in1=e0)
                den2 = cs.tile([4, CS], BF16, tag="erow", bufs=8)
                nc.vector.tensor_add(out=den2, in0=den, in1=dbp[0:4, :])
                rd = cs.tile([4, CS], BF16, tag="erow", bufs=8)
                nc.vector.reciprocal(rd, den2)
                p1 = cs.tile([4, CS], F16, tag="prow", bufs=4)
                p0 = cs.tile([4, CS], F16, tag="prow", bufs=4)
                nc.gpsimd.tensor_mul(out=p1, in0=e1, in1=rd)
                nc.gpsimd.tensor_mul(out=p0, in0=e0, in1=rd)

                # --- broadcasts + PV + combine ---
                attn = [ch.tile([128, CS], F16, tag="attn",
                                name=f"attn{c}_{dt}") for dt in range(2)]
                for dt in range(2):
                    io = dt * 128
                    p1b = psB.tile([128, CS], F32, tag="psb")
                    nc.tensor.matmul(p1b, ind_t[:, io:io + 128], p1,
                                     start=True, stop=True)
                    p0b = psB.tile([128, CS], F32, tag="psb")
                    nc.tensor.matmul(p0b, ind_t[:, io:io + 128], p0,
                                     start=True, stop=True)
                    rdb = psB.tile([128, CS], F32, tag="psb")
                    nc.tensor.matmul(rdb, indb_t[:, io:io + 128], rd,
                                     start=True, stop=True)
                    pv = psB.tile([128, CS], F32, tag="psb")
                    for hh in range(2):
                        nc.tensor.matmul(
                            pv[hh * 64:(hh + 1) * 64, :],
                            vbT[dt][hh * 64:hh * 64 + 48, :],
                            embd[dt][hh * 64:hh * 64 + 48, :],
                            start=True, stop=True,
                            tile_position=(hh * 64, hh * 64))
                    # SBUF staging: p1b/p0b/pvs on ACT; m3 takes rdb from PSUM
                    p1s = ch.tile([128, CS], F16, tag="pbs", bufs=9)
                    nc.scalar.activation(p1s, p1b, AF.Copy)
                    p0s = ch.tile([128, CS], F16, tag="pbs", bufs=9)
                    nc.scalar.activation(p0s, p0b, AF.Copy)
                    pvs = ch.tile([128, CS], BF16, tag="pbs", bufs=9)
                    nc.scalar.activation(pvs, pv, AF.Copy)
                    m1 = ch.tile([128, CS], F16, tag="mt", bufs=12)
                    nc.vector.tensor_mul(out=m1, in0=v_t[dt][:, chs], in1=p1s)
                    m2 = ch.tile([128, CS], F16, tag="mt", bufs=12)
                    if c == 0:
                        nc.vector.memset(m2[:, 0:1], 0.0)
                        nc.vector.tensor_mul(out=m2[:, 1:CS],
                                             in0=v_t[dt][:, 0:CS - 1],
                                             in1=p0s[:, 1:CS])
                    else:
                        nc.vector.tensor_mul(out=m2,
                                             in0=v_t[dt][:, cl - 1:cl + CS - 1],
                                             in1=p0s)
                    m3 = ch.tile([128, CS], F16, tag="mt", bufs=12)
                    nc.vector.tensor_mul(out=m3, in0=pvs, in1=rdb)
                    m4 = ch.tile([128, CS], F16, tag="mt", bufs=12)
                    nc.vector.tensor_add(out=m4, in0=m1, in1=m2)
                    nc.vector.tensor_add(out=attn[dt], in0=m4, in1=m3)

                # --- output projection ---
                for m in range(8):
                    acc = psO.tile([128, CS], F32, tag="oacc",
                                   name=f"oacc{c}_{m}")
                    for k in range(2):
                        nc.tensor.matmul(acc,
                                         wot_t[:, k * D + m * 128:
                                               k * D + (m + 1) * 128],
                                         attn[k], start=(k == 0), stop=(k == 1))
                    stage = ch.tile([128, CS], F16, tag="ostage", bufs=8,
                                    name=f"ost{c}_{m}")
                    if m % 2 == 0:
                        nc.scalar.activation(stage, acc, AF.Copy)
                    else:
                        nc.vector.tensor_copy(out=stage, in_=acc)
                    nc.scalar.dma_start(out=outp[m * 128:(m + 1) * 128, chs],
                                        in_=stage)
    if cap_waits:
        _cap_sync_waits(nc)
    return nc


# ---------------- host side ----------------

def _host_consts(fc):
    C = np.zeros((128, S), np.float32)
    Sg = np.zeros((128, S), np.float32)
    for p in range(128):
        i = (p % 64) // 2
        if p % 2 == 0:
            C[p] = fc[:, i, 0, 0]
            Sg[p] = fc[:, i, 0, 1]
        else:
            C[p] = fc[:, i, 1, 1]
            Sg[p] = fc[:, i, 1, 0]
    starts = np.concatenate([[0], BND[:-1] + 1])
    nrs1 = np.ones(S, np.float32)
    nrs1[starts] = 0.0
    nrs = np.broadcast_to(nrs1, (128, S)).astype(np.float16).copy()
    # constant one-step rotation R(-delta): kd[2i] = cos k[2i] + sin k[2i+1]
    inv = (1.0 / (10000.0 ** (np.arange(0, HD, 2, dtype=np.float64) / HD)))
    rotc = np.zeros((128, 2), np.float32)
    for p in range(128):
        i = (p % 64) // 2
        rotc[p, 0] = np.cos(inv[i])
        rotc[p, 1] = np.sin(inv[i]) * (1.0 if p % 2 == 0 else -1.0)
    # fp16 const blob
    cb16 = np.zeros((128, CB16_W), np.float32)
    cb16[:, CB_CBC:CB_CBC + S] = C
    cb16[:, CB_SBC:CB_SBC + S] = Sg
    cb16[:, CB_CBCB + 0:CB_CBCB + NBR] = C[:, BND]
    cb16[:, CB_SBCB + 0:CB_SBCB + NBR] = Sg[:, BND]
    for i, (dt, pi) in enumerate(((0, 0), (0, 1), (1, 0), (1, 1))):
        base = CB_OBLK + 36 * i + 32 * pi + 2 * dt
        cb16[0:64, base + 0] = 1.0
        cb16[64:128, base + 1] = 1.0
    for p in range(128):
        cb16[p, CB_PERM + (p ^ 1)] = 1.0
        cb16[p, CB_IDENT + p] = 1.0
    ind = np.zeros((4, 256), np.float32)
    ind[0, 0:64] = 1.0
    ind[1, 64:128] = 1.0
    ind[2, 128:192] = 1.0
    ind[3, 192:256] = 1.0
    cb16[0:4, CB_IND:CB_IND + 256] = ind
    # bf16 blob
    cbbf = np.zeros((128, BB_W), np.float32)
    t = np.arange(S)
    for hh in range(2):
        for jb in range(NBR):
            cbbf[hh * 64 + jb, BB_MASK:BB_MASK + S] = (
                t >= BND[jb] + 2).astype(np.float32)
    cbbf[0:48, BB_OBV + 0] = 1.0
    cbbf[64:112, BB_OBV + 1] = 1.0
    cbbf[0:48, BB_OBV + 6] = 1.0
    cbbf[64:112, BB_OBV + 7] = 1.0
    cbbf[0:4, BB_INDB:BB_INDB + 256] = ind
    return nrs, rotc, cb16, cbbf


_prog = None


def make_in_maps(x, fc, wq_, wo_, a_k_, a_v_):
    nrs, rotc, cb16, cbbf = _host_consts(fc)
    import ml_dtypes
    bf = ml_dtypes.bfloat16
    cb16_h = cb16.astype(np.float16)
    cbbf_h = cbbf.astype(bf)
    in_maps, metas = [], []
    for b in range(B):
        xT = np.ascontiguousarray(x[b].T)
        for g in range(NG):
            c0 = g * CH
            perm = np.concatenate([np.arange(c0, c0 + CH),
                                   np.arange(0, c0),
                                   np.arange(c0 + CH, D)]).astype(np.int64)
            xt_core = np.ascontiguousarray(xT[perm]).astype(np.float16)
            wq_core = wq_[c0:c0 + CH, :].T[perm]          # [D, CH]
            wqt_pack = np.ascontiguousarray(
                wq_core.reshape(8, 128, CH).transpose(1, 0, 2).reshape(
                    128, 8 * CH)).astype(np.float16)
            wo_core = wo_[:, c0:c0 + CH].T                # [CH, D]
            wot_pack = np.ascontiguousarray(
                wo_core.reshape(2, 128, D).transpose(1, 0, 2).reshape(
                    128, 2 * D)).astype(np.float16)
            apar = np.stack([a_k_[c0:c0 + 128], a_k_[c0 + 128:c0 + 256],
                             a_v_[c0:c0 + 128], a_v_[c0 + 128:c0 + 256]],
                            axis=1).astype(np.float32)
            cb32_h = np.concatenate([apar, rotc], axis=1).astype(np.float32)
            in_maps.append({
                "xt": xt_core, "wqt": wqt_pack, "wot": wot_pack,
                "nrs": nrs, "cb16": cb16_h, "cb32": cb32_h, "cbbf": cbbf_h,
            })
            metas.append((b, g))
    return in_maps, metas


def kernel(x, freq_cis, wq, wo, a_k, a_v):
    global _prog
    x = np.asarray(x, np.float32)
    fc = np.asarray(freq_cis, np.float32)
    wq_ = np.asarray(wq, np.float32)
    wo_ = np.asarray(wo, np.float32)
    a_k_ = np.asarray(a_k, np.float32)
    a_v_ = np.asarray(a_v, np.float32)
    in_maps, metas = make_in_maps(x, fc, wq_, wo_, a_k_, a_v_)
    if _prog is None:
        _prog = build_program()
    res = run_bass_kernel_spmd(_prog, in_maps, core_ids=list(range(8)))
    out = np.zeros((B, S, D), np.float32)
    for (b, g), r in zip(metas, res.results):
        out[b] += np.asarray(r["outp"], np.float32).T
    return out


if __name__ == "__main__":
    build_program()
    print("program built ok")


# revision 5
# speedup vs baseline: 1.0092x; 1.0018x over previous
"""Trainium2 Bass kernel for AttentiveSSMNoProjCyc (sparse_attention), v2.

Sharding: 8 cores = 2 batches x 4 head-groups (4 heads / 256 channels each).
Per core, [channel, time] layout, fp16 compute domain / bf16 exp domain:
  - SSM scans via one fused tensor_tensor_scan [128, 2*S] per d-tile
  - band scores use rope cancellation: diag = q.k, sub-diag = q.(R(-d)k)
    with a constant per-partition rotation (no full k-rope)
  - boundary keys: gather 33 cols of k, rope them at [128,48] cost
  - q-rope via PE permutation matmul + ACT copy + DVE/Pool fp16 TT
  - per-chunk softmax / combine pipeline
  - DMAs consolidated into packed blobs (HWDGE generation is serialized)
Host sums the 4 per-batch partials and transposes back.
"""
import numpy as np

import concourse.bass as bass
import concourse.mybir as mybir
from concourse.bass_utils import run_bass_kernel_spmd
from concourse.tile import TileContext
import concourse.tile as _tile_mod
from concourse.vector_clock import ScopedClock as _ScopedClock


def _split_drain_and_barrier(self, tick_clock, wait_clock):
    """Tail drain, with its sem waits spread over chained SP nops."""
    probe = self.nc.sync.nop()
    wait_clock.add_sem_waits(
        probe.ins, _ScopedClock({None: tick_clock.global_clock})
    )
    si = probe.ins.sync_info
    waits = list(si.on_wait) if si is not None else []
    upds = list(si.on_update) if si is not None else []
    MAXW = 1
    if len(waits) > MAXW:
        probe.ins.sync_info = mybir.SyncInfo(on_wait=waits[:MAXW],
                                             on_update=upds)
        for i in range(MAXW, len(waits), MAXW):
            extra = self.nc.sync.nop()
            extra.ins.sync_info = mybir.SyncInfo(
                on_wait=waits[i:i + MAXW], on_update=[])
    self.nc.sync.drain()

    self.nc.all_engine_barrier()
    assert self.sems is not None
    popped = self.nc._tile_sem_poison_stack.pop()
    assert popped is self._sem_poison
    self.nc.clear_and_free_semaphores(list(self.sems.allocated().values()))
    self.nc.all_engine_barrier()


_tile_mod.TileContext._drain_and_barrier = _split_drain_and_barrier


def _cap_sync_waits(nc, cap=1):
    """Hoist excess sync waits onto same-engine carrier NOPs."""
    nid = [0]

    def mknop(engine, waits):
        nid[0] += 1
        nop = mybir.InstNoOp(name=f"I-capw-{nid[0]}", ins=[], outs=[])
        nop.engine = engine
        nop.sync_info = mybir.SyncInfo(on_wait=list(waits), on_update=[])
        return nop

    for bb in nc.m.functions[0].blocks:
        il = bb.instructions
        i = 0
        while i < len(il):
            ins = il[i]
            si = ins.sync_info
            nw = len(si.on_wait) if si is not None else 0
            if nw > cap:
                waits = list(si.on_wait)
                ins.sync_info = mybir.SyncInfo(on_wait=waits[:cap],
                                               on_update=list(si.on_update))
                rest = waits[cap:]
                pos = i
                for j in range(0, len(rest), cap):
                    il.insert(pos, mknop(ins.engine, rest[j:j + cap]))
                    pos += 1
                    i += 1
            i += 1


B, S, D, H, HD = 2, 2048, 1024, 16, 64
NG = 4            # head-groups per batch
CH = 256          # channels per core (4 heads)
NB = 48           # padded boundary columns (33 real)
NBD = 112         # blockdiag boundary cols: head0 -> 0:48, head1 -> 64:112
NCHUNK = 4
CS = S // NCHUNK  # 512
F32 = mybir.dt.float32
F16 = mybir.dt.float16
BF16 = mybir.dt.bfloat16
AL = mybir.AluOpType
AF = mybir.ActivationFunctionType
NEG = -1e30

SHUF_XOR1 = [i ^ 1 for i in range(32)]

# packed fp16 const blob layout (columns)
CB_CBC = 0                 # [128, S]
CB_SBC = CB_CBC + S        # [128, S]
CB_CBCB = CB_SBC + S       # [128, NB]
CB_SBCB = CB_CBCB + NB     # [128, NB]
CB_OBLK = CB_SBCB + NB     # [128, 144]: 4x 36-col variants
CB_PERM = CB_OBLK + 144    # [128, 128]
CB_IDENT = CB_PERM + 128   # [128, 128]
CB_IND = CB_IDENT + 128    # [4, 256]
CB16_W = CB_IND + 256

# packed bf16 blob layout
BB_MASK = 0                # [NBD, S]
BB_OBV = BB_MASK + S       # [NBD, 8]
BB_INDB = BB_OBV + 8       # [4, 256]
BB_W = BB_INDB + 256

# packed fp32 blob: apar [128,4] | rotc [128,2]
C32_W = 6


def _boundaries():
    K_, LAYER_, NLAYERS_, MAXLEN_ = 64, 4, 16, 16384
    off = min(K_ - 1, LAYER_ * (K_ // NLAYERS_))
    bl = [b - off for b in range(K_ - 1, MAXLEN_, K_)]
    if bl[-1] != MAXLEN_ - 1:
        bl.append(MAXLEN_ - 1)
    if bl[0] != 0:
        bl.insert(0, 0)
    b = np.asarray(bl)
    b = b[b < S].copy()
    b[-1] = S - 1
    return b


BND = _boundaries()
NBR = len(BND)  # 33


def build_program(cap_waits=True):
    nc = bass.Bass()
    dp = nc.declare_dram_parameter
    xt = dp("xt", [D, S], F16, isOutput=False)
    wqt = dp("wqt", [128, 8 * CH], F16, isOutput=False)   # packed k-tiles
    wot = dp("wot", [128, 2 * D], F16, isOutput=False)    # packed k-tiles
    nrs = dp("nrs", [128, S], F16, isOutput=False)
    cb16 = dp("cb16", [128, CB16_W], F16, isOutput=False)
    cb32 = dp("cb32", [128, C32_W], F32, isOutput=False)
    cbbf = dp("cbbf", [128, BB_W], BF16, isOutput=False)
    outp = dp("outp", [D, S], F16, isOutput=True)

    with TileContext(nc) as tc, nc.allow_low_precision(
            reason="2e-2 output tolerance; fp16/bf16 validated vs reference"):
        with (
            tc.tile_pool(name="persist", bufs=1) as pp,
            tc.tile_pool(name="scanio", bufs=3) as sio,     # A4/B4/kv 8K tiles
            tc.tile_pool(name="chk", bufs=3) as ch,         # per-chunk f16 tiles
            tc.tile_pool(name="chsm", bufs=3) as cs,        # per-chunk small rows
            tc.tile_pool(name="small", bufs=2) as ck,
            tc.tile_pool(name="psQ", bufs=2, space="PSUM") as psQ,
            tc.tile_pool(name="psB", bufs=4, space="PSUM") as psB,
            tc.tile_pool(name="psT", bufs=1, space="PSUM") as psT,
            tc.tile_pool(name="psO", bufs=2, space="PSUM") as psO,
        ):
            # ============ DMAs (ordered: scan-critical first) ============
            x_t = [pp.tile([128, S], F16, tag=f"x{k}", name=f"x_t{k}")
                   for k in range(8)]
            for k in range(2):
                nc.sync.dma_start(out=x_t[k], in_=xt[k * 128:(k + 1) * 128, :])
            nrs_t = pp.tile([128, S], F16, tag="nrs")
            nc.sync.dma_start(out=nrs_t, in_=nrs[:, :])
            cb32_t = pp.tile([128, C32_W], F32, tag="cb32")
            nc.sync.dma_start(out=cb32_t, in_=cb32[:, :])
            cb16_t = pp.tile([128, CB16_W], F16, tag="cb16")
            nc.sync.dma_start(out=cb16_t, in_=cb16[:, :])
            wqt_t = pp.tile([128, 8 * CH], F16, tag="wqt")
            nc.sync.dma_start(out=wqt_t, in_=wqt[:, :])
            for k in range(2, 8):
                nc.sync.dma_start(out=x_t[k], in_=xt[k * 128:(k + 1) * 128, :])
            wot_t = pp.tile([128, 2 * D], F16, tag="wot")
            nc.sync.dma_start(out=wot_t, in_=wot[:, :])
            cbbf_t = pp.tile([128, BB_W], BF16, tag="cbbf")
            nc.sync.dma_start(out=cbbf_t, in_=cbbf[:, :])

            apar_t = cb32_t[:, 0:4]
            cosP = cb32_t[:, 4:5]
            sinPs = cb32_t[:, 5:6]
            cbc_t = cb16_t[:, CB_CBC:CB_CBC + S]
            sbc_t = cb16_t[:, CB_SBC:CB_SBC + S]
            cbcb_t = cb16_t[:, CB_CBCB:CB_CBCB + NB]
            sbcb_t = cb16_t[:, CB_SBCB:CB_SBCB + NB]
            oblk_t = cb16_t[:, CB_OBLK:CB_OBLK + 144]
            permw_t = cb16_t[:, CB_PERM:CB_PERM + 128]
            ident_t = cb16_t[:, CB_IDENT:CB_IDENT + 128]
            ind_t = cb16_t[0:4, CB_IND:CB_IND + 256]
            maskb_t = cbbf_t[0:NBD, BB_MASK:BB_MASK + S]
            obv_t = cbbf_t[0:NBD, BB_OBV:BB_OBV + 8]
            indb_t = cbbf_t[0:4, BB_INDB:BB_INDB + 256]

            asig = pp.tile([128, 4], F32, tag="asig")
            nc.scalar.activation(asig, apar_t, AF.Sigmoid)
            oma = pp.tile([128, 4], F32, tag="oma")  # 1 - sigmoid(a)
            nc.vector.tensor_scalar(out=oma, in0=asig, scalar1=-1.0,
                                    scalar2=1.0, op0=AL.mult, op1=AL.add)

            # ============ SSM scans (k and v fused per d-tile) ============
            kpre = [pp.tile([128, S], F16, tag=f"kp{dt}", name=f"kpre{dt}")
                    for dt in range(2)]
            v_t = [pp.tile([128, S], F16, tag=f"v{dt}", name=f"v_t{dt}")
                   for dt in range(2)]
            for dt in range(2):
                A4 = sio.tile([128, 2 * S], F16, tag="sc8k")
                B4 = sio.tile([128, 2 * S], F16, tag="sc8k")
                nc.vector.tensor_scalar(out=A4[:, 0:S], in0=nrs_t,
                                        scalar1=asig[:, dt:dt + 1],
                                        scalar2=None, op0=AL.mult)
                nc.vector.tensor_scalar(out=A4[:, S:2 * S], in0=nrs_t,
                                        scalar1=asig[:, 2 + dt:3 + dt],
                                        scalar2=None, op0=AL.mult)
                nc.vector.tensor_scalar(out=B4[:, 0:S], in0=x_t[dt],
                                        scalar1=oma[:, dt:dt + 1],
                                        scalar2=None, op0=AL.mult)
                nc.vector.tensor_scalar(out=B4[:, S:2 * S], in0=x_t[dt],
                                        scalar1=oma[:, 2 + dt:3 + dt],
                                        scalar2=None, op0=AL.mult)
                kv = sio.tile([128, 2 * S], F16, tag="sc8k")
                nc.vector.tensor_tensor_scan(out=kv, data0=A4, data1=B4,
                                             initial=0.0, op0=AL.mult,
                                             op1=AL.add)
                nc.vector.tensor_add(out=kpre[dt], in0=kv[:, 0:S],
                                     in1=x_t[dt])
                nc.vector.tensor_add(out=v_t[dt], in0=kv[:, S:2 * S],
                                     in1=x_t[dt])

            # ============ kd: constant rotation of kpre (for sub-diag) ======
            # kd = cosP*k + sinPs*shuffle(k); shuffle via PE permutation.
            kd = [pp.tile([128, S], F16, tag=f"kd{dt}", name=f"kd{dt}")
                  for dt in range(2)]
            for dt in range(2):
                t2k = sio.tile([128, S], F16, tag="t2k", bufs=1)
                for c in range(NCHUNK):
                    chs = slice(c * CS, (c + 1) * CS)
                    zp = psB.tile([128, CS], F32, tag="psb")
                    nc.tensor.matmul(zp, permw_t, kpre[dt][:, chs],
                                     start=True, stop=True)
                    nc.scalar.activation(t2k[:, chs], zp, AF.Copy,
                                         scale=sinPs)
                t1k = sio.tile([128, S], F16, tag="t1k", bufs=1)
                nc.vector.tensor_scalar(out=t1k, in0=kpre[dt], scalar1=cosP,
                                        scalar2=None, op0=AL.mult)
                nc.vector.tensor_add(out=kd[dt], in0=t1k, in1=t2k)

            # ============ boundary keys (gather 33 cols, rope there) =======
            kbd = [pp.tile([128, NBD], F16, tag=f"kbd{dt}", name=f"kbd{dt}")
                   for dt in range(2)]
            vbT = [pp.tile([128, 64], BF16, tag=f"vbT{dt}", name=f"vbT{dt}")
                   for dt in range(2)]
            for dt in range(2):
                kb = ck.tile([128, NB], F16, tag="kb")
                vb = ck.tile([128, NB], F16, tag="vb")
                for src_t, dst_t in ((kpre[dt], kb), (v_t[dt], vb)):
                    nc.vector.memset(dst_t[:, 33:NB], 0.0)
                    nc.vector.tensor_copy(out=dst_t[:, 0:1], in_=src_t[:, 0:1])
                    nc.vector.tensor_copy(
                        out=dst_t[:, 1:32],
                        in_=src_t.rearrange("p (a b) -> p a b", b=64)[:, 0:31, 47])
                    nc.vector.tensor_copy(out=dst_t[:, 32:33],
                                          in_=src_t[:, S - 1:S])
                # rope kb at boundary positions
                zb = ck.tile([128, NB], F16, tag="zb")
                nc.vector.stream_shuffle(zb, kb, SHUF_XOR1)
                t1b = ck.tile([128, NB], F16, tag="t1b")
                nc.vector.tensor_mul(out=t1b, in0=kb, in1=cbcb_t)
                t2b = ck.tile([128, NB], F16, tag="t2b")
                nc.vector.tensor_mul(out=t2b, in0=zb, in1=sbcb_t)
                krb = ck.tile([128, NB], F16, tag="krb")
                nc.vector.tensor_add(out=krb, in0=t1b, in1=t2b)
                # blockdiag [128, NBD]
                nc.vector.memset(kbd[dt], 0.0)
                nc.vector.tensor_copy(out=kbd[dt][0:64, 0:48], in_=krb[0:64, :])
                nc.vector.tensor_copy(out=kbd[dt][64:128, 64:112],
                                      in_=krb[64:128, :])
                # vbT: transpose vb blocks -> [48, 64] per head-half
                for hh in range(2):
                    tp = psT.tile([128, CS // 2], F16, tag="psbh",
                                  name=f"tp{dt}_{hh}")
                    nc.tensor.transpose(tp[0:48, 0:64],
                                        vb[hh * 64:(hh + 1) * 64, 0:48],
                                        ident_t[hh * 64:(hh + 1) * 64,
                                                hh * 64:(hh + 1) * 64],
                                        tile_position=(hh * 64, 0))
                    nc.scalar.activation(vbT[dt][hh * 64:hh * 64 + 48, :],
                                         tp[0:48, 0:64], AF.Copy)

            # ============ main per-chunk pipeline ============
            for c in range(NCHUNK):
                cl = c * CS
                chs = slice(cl, cl + CS)
                # --- Q projection ---
                accs = [psQ.tile([128, CS], F32, tag="mmacc",
                                 name=f"qacc{c}_{m}") for m in range(2)]
                for m in range(2):
                    for k in range(8):
                        nc.tensor.matmul(accs[m],
                                         wqt_t[:, k * CH + m * 128:
                                               k * CH + (m + 1) * 128],
                                         x_t[k][:, chs],
                                         start=(k == 0), stop=(k == 7))
                xq = [ch.tile([128, CS], F16, tag="xq", name=f"xq{c}_{dt}")
                      for dt in range(2)]
                for dt in range(2):
                    nc.scalar.activation(xq[dt], accs[dt], AF.Copy)
                # --- q rope (perm via PE; muls DVE+Pool) ---
                qr = [ch.tile([128, CS], F16, tag="qr", name=f"qr{c}_{dt}")
                      for dt in range(2)]
                for dt in range(2):
                    zq = psB.tile([128, CS], F32, tag="psb")
                    nc.tensor.matmul(zq, permw_t, xq[dt], start=True, stop=True)
                    zqs = ch.tile([128, CS], F16, tag="zqs")
                    nc.scalar.activation(zqs, zq, AF.Copy)
                    tq1 = ch.tile([128, CS], F16, tag="tq1")
                    nc.vector.tensor_mul(out=tq1, in0=xq[dt],
                                         in1=cbc_t[:, chs])
                    tq2 = ch.tile([128, CS], F16, tag="tq2")
                    nc.gpsimd.tensor_mul(out=tq2, in0=zqs, in1=sbc_t[:, chs])
                    nc.vector.tensor_add(out=qr[dt], in0=tq1, in1=tq2)

                # --- band scores (dt-accumulated, s1 rows 0:4 / s0 rows 4:8) --
                sp = psB.tile([128, CS], F32, tag="psb")
                prods = []
                for dt in range(2):
                    p1t = ch.tile([128, CS], F16, tag="prod", bufs=4)
                    nc.vector.tensor_mul(out=p1t, in0=xq[dt],
                                         in1=kpre[dt][:, chs])
                    p0t = ch.tile([128, CS], F16, tag="prod", bufs=4)
                    if c == 0:
                        nc.vector.memset(p0t[:, 0:1], 0.0)
                        nc.vector.tensor_mul(out=p0t[:, 1:CS],
                                             in0=xq[dt][:, 1:CS],
                                             in1=kd[dt][:, 0:CS - 1])
                    else:
                        nc.vector.tensor_mul(out=p0t, in0=xq[dt],
                                             in1=kd[dt][:, cl - 1:cl + CS - 1])
                    prods.append((p1t, p0t))
                for i, (dt, pi) in enumerate(((0, 0), (0, 1),
                                              (1, 0), (1, 1))):
                    nc.tensor.matmul(sp[0:36, :],
                                     oblk_t[:, 36 * i:36 * i + 36],
                                     prods[dt][pi], start=(i == 0),
                                     stop=(i == 3))
                if c == 0:
                    nc.vector.memset(sp[32:36, 0:1], NEG)
                e1 = cs.tile([4, CS], BF16, tag="erow", bufs=8, name=f"e1_{c}")
                nc.scalar.activation(e1, sp[0:4, :], AF.Exp, scale=0.125)
                e0 = cs.tile([4, CS], BF16, tag="erow", bufs=8, name=f"e0_{c}")
                nc.scalar.activation(e0, sp[32:36, :], AF.Exp, scale=0.125)

                # --- boundary scores ---
                embd = [ch.tile([NBD, CS], BF16, tag="embd", bufs=6,
                                name=f"embd{c}_{dt}") for dt in range(2)]
                for dt in range(2):
                    sb = psB.tile([128, CS], F32, tag="psb")
                    nc.tensor.matmul(sb[0:NBD, :], kbd[dt], qr[dt],
                                     start=True, stop=True)
                    eb = ch.tile([NBD, CS], BF16, tag="ebx")
                    nc.scalar.activation(eb, sb[0:NBD, :], AF.Exp, scale=0.125)
                    nc.gpsimd.tensor_mul(out=embd[dt], in0=eb,
                                         in1=maskb_t[:, chs])

                # --- denominator / p rows ---
                dbp = psB.tile([128, CS], F32, tag="psb")
                for dt in range(2):
                    nc.tensor.matmul(dbp[0:4, :], obv_t[:, 4 * dt:4 * dt + 4],
                                     embd[dt], start=(dt == 0), stop=(dt == 1))
                den = cs.tile([4, CS], BF16, tag="erow", bufs=8)
                nc.vector.tensor_add(out=den, in0=e1, in1=e0)
                den2 = cs.tile([4, CS], BF16, tag="erow", bufs=8)
                nc.vector.tensor_add(out=den2, in0=den, in1=dbp[0:4, :])
                rd = cs.tile([4, CS], BF16, tag="erow", bufs=8)
                nc.vector.reciprocal(rd, den2)
                p1 = cs.tile([4, CS], F16, tag="prow", bufs=4)
                p0 = cs.tile([4, CS], F16, tag="prow", bufs=4)
                nc.gpsimd.tensor_mul(out=p1, in0=e1, in1=rd)
                nc.gpsimd.tensor_mul(out=p0, in0=e0, in1=rd)

                # --- broadcasts + PV + combine ---
                attn = [ch.tile([128, CS], F16, tag="attn",
                                name=f"attn{c}_{dt}") for dt in range(2)]
                for dt in range(2):
                    io = dt * 128
                    p1b = psB.tile([128, CS], F32, tag="psb")
                    nc.tensor.matmul(p1b, ind_t[:, io:io + 128], p1,
                                     start=True, stop=True)
                    p0b = psB.tile([128, CS], F32, tag="psb")
                    nc.tensor.matmul(p0b, ind_t[:, io:io + 128], p0,
                                     start=True, stop=True)
                    rdb = psB.tile([128, CS], F32, tag="psb")
                    nc.tensor.matmul(rdb, indb_t[:, io:io + 128], rd,
                                     start=True, stop=True)
                    pv = psB.tile([128, CS], F32, tag="psb")
                    for hh in range(2):
                        nc.tensor.matmul(
                            pv[hh * 64:(hh + 1) * 64, :],
                            vbT[dt][hh * 64:hh * 64 + 48, :],
                            embd[dt][hh * 64:hh * 64 + 48, :],
                            start=True, stop=True,
                            tile_position=(hh * 64, hh * 64))
                    # SBUF staging: p1b/p0b/pvs on ACT; m3 takes rdb from PSUM
                    p1s = ch.tile([128, CS], F16, tag="pbs", bufs=9)
                    nc.scalar.activation(p1s, p1b, AF.Copy)
                    p0s = ch.tile([128, CS], F16, tag="pbs", bufs=9)
                    nc.scalar.activation(p0s, p0b, AF.Copy)
                    pvs = ch.tile([128, CS], BF16, tag="pbs", bufs=9)
                    nc.scalar.activation(pvs, pv, AF.Copy)
                    m1 = ch.tile([128, CS], F16, tag="mt", bufs=12)
                    nc.vector.tensor_mul(out=m1, in0=v_t[dt][:, chs], in1=p1s)
                    m2 = ch.tile([128, CS], F16, tag="mt", bufs=12)
                    if c == 0:
                        nc.vector.memset(m2[:, 0:1], 0.0)
                        nc.vector.tensor_mul(out=m2[:, 1:CS],
                                             in0=v_t[dt][:, 0:CS - 1],
                                             in1=p0s[:, 1:CS])
                    else:
                        nc.vector.tensor_mul(out=m2,
                                             in0=v_t[dt][:, cl - 1:cl + CS - 1],
                                             in1=p0s)
                    m3 = ch.tile([128, CS], F16, tag="mt", bufs=12)
                    nc.vector.tensor_mul(out=m3, in0=pvs, in1=rdb)
                    m4 = ch.tile([128, CS], F16, tag="mt", bufs=12)
                    nc.vector.tensor_add(out=m4, in0=m1, in1=m2)
                    nc.vector.tensor_add(out=attn[dt], in0=m4, in1=m3)

                # --- output projection ---
                for m in range(8):
                    acc = psO.tile([128, CS], F32, tag="oacc",
                                   name=f"oacc{c}_{m}")
                    for k in range(2):
                        nc.tensor.matmul(acc,
                                         wot_t[:, k * D + m * 128:
                                               k * D + (m + 1) * 128],
                                         attn[k], start=(k == 0), stop=(k == 1))
                    stage = ch.tile([128, CS], F16, tag="ostage", bufs=8,
                                    name=f"ost{c}_{m}")
                    if m % 2 == 0:
                        nc.scalar.activation(stage, acc, AF.Copy)
                    else:
                        nc.vector.tensor_copy(out=stage, in_=acc)
                    nc.scalar.dma_start(out=outp[m * 128:(m + 1) * 128, chs],
                                        in_=stage)
    if cap_waits:
        _cap_sync_waits(nc)
    return nc


# ---------------- host side ----------------

def _host_consts(fc):
    C = np.zeros((128, S), np.float32)
    Sg = np.zeros((128, S), np.float32)
    for p in range(128):
        i = (p % 64) // 2
        if p % 2 == 0:
            C[p] = fc[:, i, 0, 0]
            Sg[p] = fc[:, i, 0, 1]
        else:
            C[p] = fc[:, i, 1, 1]
            Sg[p] = fc[:, i, 1, 0]
    starts = np.concatenate([[0], BND[:-1] + 1])
    nrs1 = np.ones(S, np.float32)
    nrs1[starts] = 0.0
    nrs = np.broadcast_to(nrs1, (128, S)).astype(np.float16).copy()
    # constant one-step rotation R(-delta): kd[2i] = cos k[2i] + sin k[2i+1]
    inv = (1.0 / (10000.0 ** (np.arange(0, HD, 2, dtype=np.float64) / HD)))
    rotc = np.zeros((128, 2), np.float32)
    for p in range(128):
        i = (p % 64) // 2
        rotc[p, 0] = np.cos(inv[i])
        rotc[p, 1] = np.sin(inv[i]) * (1.0 if p % 2 == 0 else -1.0)
    # fp16 const blob
    cb16 = np.zeros((128, CB16_W), np.float32)
    cb16[:, CB_CBC:CB_CBC + S] = C
    cb16[:, CB_SBC:CB_SBC + S] = Sg
    cb16[:, CB_CBCB + 0:CB_CBCB + NBR] = C[:, BND]
    cb16[:, CB_SBCB + 0:CB_SBCB + NBR] = Sg[:, BND]
    for i, (dt, pi) in enumerate(((0, 0), (0, 1), (1, 0), (1, 1))):
        base = CB_OBLK + 36 * i + 32 * pi + 2 * dt
        cb16[0:64, base + 0] = 1.0
        cb16[64:128, base + 1] = 1.0
    for p in range(128):
        cb16[p, CB_PERM + (p ^ 1)] = 1.0
        cb16[p, CB_IDENT + p] = 1.0
    ind = np.zeros((4, 256), np.float32)
    ind[0, 0:64] = 1.0
    ind[1, 64:128] = 1.0
    ind[2, 128:192] = 1.0
    ind[3, 192:256] = 1.0
    cb16[0:4, CB_IND:CB_IND + 256] = ind
    # bf16 blob
    cbbf = np.zeros((128, BB_W), np.float32)
    t = np.arange(S)
    for hh in range(2):
        for jb in range(NBR):
            cbbf[hh * 64 + jb, BB_MASK:BB_MASK + S] = (
                t >= BND[jb] + 2).astype(np.float32)
    cbbf[0:48, BB_OBV + 0] = 1.0
    cbbf[64:112, BB_OBV + 1] = 1.0
    cbbf[0:48, BB_OBV + 6] = 1.0
    cbbf[64:112, BB_OBV + 7] = 1.0
    cbbf[0:4, BB_INDB:BB_INDB + 256] = ind
    return nrs, rotc, cb16, cbbf


_prog = None


def make_in_maps(x, fc, wq_, wo_, a_k_, a_v_):
    nrs, rotc, cb16, cbbf = _host_consts(fc)
    import ml_dtypes
    bf = ml_dtypes.bfloat16
    cb16_h = cb16.astype(np.float16)
    cbbf_h = cbbf.astype(bf)
    in_maps, metas = [], []
    for b in range(B):
        xT = np.ascontiguousarray(x[b].T)
        for g in range(NG):
            c0 = g * CH
            perm = np.concatenate([np.arange(c0, c0 + CH),
                                   np.arange(0, c0),
                                   np.arange(c0 + CH, D)]).astype(np.int64)
            xt_core = np.ascontiguousarray(xT[perm]).astype(np.float16)
            wq_core = wq_[c0:c0 + CH, :].T[perm]          # [D, CH]
            wqt_pack = np.ascontiguousarray(
                wq_core.reshape(8, 128, CH).transpose(1, 0, 2).reshape(
                    128, 8 * CH)).astype(np.float16)
            wo_core = wo_[:, c0:c0 + CH].T                # [CH, D]
            wot_pack = np.ascontiguousarray(
                wo_core.reshape(2, 128, D).transpose(1, 0, 2).reshape(
                    128, 2 * D)).astype(np.float16)
            apar = np.stack([a_k_[c0:c0 + 128], a_k_[c0 + 128:c0 + 256],
                             a_v_[c0:c0 + 128], a_v_[c0 + 128:c0 + 256]],
                            axis=1).astype(np.float32)
            cb32_h = np.concatenate([apar, rotc], axis=1).astype(np.float32)
            in_maps.append({
                "xt": xt_core, "wqt": wqt_pack, "wot": wot_pack,
                "nrs": nrs, "cb16": cb16_h, "cb32": cb32_h, "cbbf": cbbf_h,
            })
            metas.append((b, g))
    return in_maps, metas


def kernel(x, freq_cis, wq, wo, a_k, a_v):
    global _prog
    x = np.asarray(x, np.float32)
    fc = np.asarray(freq_cis, np.float32)
    wq_ = np.asarray(wq, np.float32)
    wo_ = np.asarray(wo, np.float32)
    a_k_ = np.asarray(a_k, np.float32)
    a_v_ = np.asarray(a_v, np.float32)
    in_maps, metas = make_in_maps(x, fc, wq_, wo_, a_k_, a_v_)
    if _prog is None:
        _prog = build_program()
    res = run_bass_kernel_spmd(_prog, in_maps, core_ids=list(range(8)))
    out = np.zeros((B, S, D), np.float32)
    for (b, g), r in zip(metas, res.results):
        out[b] += np.asarray(r["outp"], np.float32).T
    return out


if __name__ == "__main__":
    build_program()
    print("program built ok")


# revision 6
# speedup vs baseline: 1.0196x; 1.0103x over previous
"""Trainium2 Bass kernel for AttentiveSSMNoProjCyc (sparse_attention), v2.

Sharding: 8 cores = 2 batches x 4 head-groups (4 heads / 256 channels each).
Per core, [channel, time] layout, fp16 compute domain / bf16 exp domain:
  - SSM scans via one fused tensor_tensor_scan [128, 2*S] per d-tile
  - band scores use rope cancellation: diag = q.k, sub-diag = q.(R(-d)k)
    with a constant per-partition rotation (no full k-rope)
  - boundary keys: gather 33 cols of k, rope them at [128,48] cost
  - q-rope via PE permutation matmul + ACT copy + DVE/Pool fp16 TT
  - per-chunk softmax / combine pipeline
  - DMAs consolidated into packed blobs (HWDGE generation is serialized)
Host sums the 4 per-batch partials and transposes back.
"""
import numpy as np

import concourse.bass as bass
import concourse.mybir as mybir
from concourse.bass_utils import run_bass_kernel_spmd
from concourse.tile import TileContext
import concourse.tile as _tile_mod
from concourse.vector_clock import ScopedClock as _ScopedClock


def _split_drain_and_barrier(self, tick_clock, wait_clock):
    """Tail drain, with its sem waits spread over chained SP nops."""
    probe = self.nc.sync.nop()
    wait_clock.add_sem_waits(
        probe.ins, _ScopedClock({None: tick_clock.global_clock})
    )
    si = probe.ins.sync_info
    waits = list(si.on_wait) if si is not None else []
    upds = list(si.on_update) if si is not None else []
    MAXW = 1
    if len(waits) > MAXW:
        probe.ins.sync_info = mybir.SyncInfo(on_wait=waits[:MAXW],
                                             on_update=upds)
        for i in range(MAXW, len(waits), MAXW):
            extra = self.nc.sync.nop()
            extra.ins.sync_info = mybir.SyncInfo(
                on_wait=waits[i:i + MAXW], on_update=[])
    self.nc.sync.drain()

    self.nc.all_engine_barrier()
    assert self.sems is not None
    popped = self.nc._tile_sem_poison_stack.pop()
    assert popped is self._sem_poison
    self.nc.clear_and_free_semaphores(list(self.sems.allocated().values()))
    self.nc.all_engine_barrier()


_tile_mod.TileContext._drain_and_barrier = _split_drain_and_barrier


def _cap_sync_waits(nc, cap=1):
    """Hoist excess sync waits onto same-engine carrier NOPs."""
    nid = [0]

    def mknop(engine, waits):
        nid[0] += 1
        nop = mybir.InstNoOp(name=f"I-capw-{nid[0]}", ins=[], outs=[])
        nop.engine = engine
        nop.sync_info = mybir.SyncInfo(on_wait=list(waits), on_update=[])
        return nop

    for bb in nc.m.functions[0].blocks:
        il = bb.instructions
        i = 0
        while i < len(il):
            ins = il[i]
            si = ins.sync_info
            nw = len(si.on_wait) if si is not None else 0
            if nw > cap:
                waits = list(si.on_wait)
                ins.sync_info = mybir.SyncInfo(on_wait=waits[:cap],
                                               on_update=list(si.on_update))
                rest = waits[cap:]
                pos = i
                for j in range(0, len(rest), cap):
                    il.insert(pos, mknop(ins.engine, rest[j:j + cap]))
                    pos += 1
                    i += 1
            i += 1


B, S, D, H, HD = 2, 2048, 1024, 16, 64
NG = 4            # head-groups per batch
CH = 256          # channels per core (4 heads)
NB = 48           # padded boundary columns (33 real)
NBD = 112         # blockdiag boundary cols: head0 -> 0:48, head1 -> 64:112
NCHUNK = 4
CS = S // NCHUNK  # 512
F32 = mybir.dt.float32
F16 = mybir.dt.float16
BF16 = mybir.dt.bfloat16
AL = mybir.AluOpType
AF = mybir.ActivationFunctionType
NEG = -1e30

SHUF_XOR1 = [i ^ 1 for i in range(32)]

# packed fp16 const blob layout (columns)
CB_CBC = 0                 # [128, S]
CB_SBC = CB_CBC + S        # [128, S]
CB_CBCB = CB_SBC + S       # [128, NB]
CB_SBCB = CB_CBCB + NB     # [128, NB]
CB_OBLK = CB_SBCB + NB     # [128, 144]: 4x 36-col variants
CB_PERM = CB_OBLK + 144    # [128, 128]
CB_IDENT = CB_PERM + 128   # [128, 128]
CB_IND = CB_IDENT + 128    # [4, 256]
CB16_W = CB_IND + 256

# packed bf16 blob layout
BB_MASK = 0                # [NBD, S]
BB_OBV = BB_MASK + S       # [NBD, 8]
BB_INDB = BB_OBV + 8       # [4, 256]
BB_W = BB_INDB + 256

# packed fp32 blob: apar [128,4] | rotc [128,2]
C32_W = 6


def _boundaries():
    K_, LAYER_, NLAYERS_, MAXLEN_ = 64, 4, 16, 16384
    off = min(K_ - 1, LAYER_ * (K_ // NLAYERS_))
    bl = [b - off for b in range(K_ - 1, MAXLEN_, K_)]
    if bl[-1] != MAXLEN_ - 1:
        bl.append(MAXLEN_ - 1)
    if bl[0] != 0:
        bl.insert(0, 0)
    b = np.asarray(bl)
    b = b[b < S].copy()
    b[-1] = S - 1
    return b


BND = _boundaries()
NBR = len(BND)  # 33


def build_program(cap_waits=True):
    nc = bass.Bass()
    dp = nc.declare_dram_parameter
    xt = dp("xt", [D, S], F16, isOutput=False)
    wqt = dp("wqt", [128, 8 * CH], F16, isOutput=False)   # packed k-tiles
    wot = dp("wot", [128, 2 * D], F16, isOutput=False)    # packed k-tiles
    nrs = dp("nrs", [128, S], F16, isOutput=False)
    cb16 = dp("cb16", [128, CB16_W], F16, isOutput=False)
    cb32 = dp("cb32", [128, C32_W], F32, isOutput=False)
    cbbf = dp("cbbf", [128, BB_W], BF16, isOutput=False)
    outp = dp("outp", [D, S], F16, isOutput=True)

    with TileContext(nc) as tc, nc.allow_low_precision(
            reason="2e-2 output tolerance; fp16/bf16 validated vs reference"):
        with (
            tc.tile_pool(name="persist", bufs=1) as pp,
            tc.tile_pool(name="scanio", bufs=3) as sio,     # A4/B4/kv 8K tiles
            tc.tile_pool(name="chk", bufs=3) as ch,         # per-chunk f16 tiles
            tc.tile_pool(name="chsm", bufs=3) as cs,        # per-chunk small rows
            tc.tile_pool(name="small", bufs=2) as ck,
            tc.tile_pool(name="psQ", bufs=2, space="PSUM") as psQ,
            tc.tile_pool(name="psB", bufs=4, space="PSUM") as psB,
            tc.tile_pool(name="psT", bufs=1, space="PSUM") as psT,
            tc.tile_pool(name="psO", bufs=2, space="PSUM") as psO,
        ):
            # ============ DMAs (ordered: scan-critical first) ============
            x_t = [pp.tile([128, S], F16, tag=f"x{k}", name=f"x_t{k}")
                   for k in range(8)]
            for k in range(2):
                nc.sync.dma_start(out=x_t[k], in_=xt[k * 128:(k + 1) * 128, :])
            nrs_t = pp.tile([128, S], F16, tag="nrs")
            nc.sync.dma_start(out=nrs_t, in_=nrs[:, :])
            cb32_t = pp.tile([128, C32_W], F32, tag="cb32")
            nc.sync.dma_start(out=cb32_t, in_=cb32[:, :])
            cb16_t = pp.tile([128, CB16_W], F16, tag="cb16")
            nc.sync.dma_start(out=cb16_t, in_=cb16[:, :])
            wqt_t = pp.tile([128, 8 * CH], F16, tag="wqt")
            nc.sync.dma_start(out=wqt_t, in_=wqt[:, :])
            for k in range(2, 8):
                nc.sync.dma_start(out=x_t[k], in_=xt[k * 128:(k + 1) * 128, :])
            wot_t = pp.tile([128, 2 * D], F16, tag="wot")
            nc.sync.dma_start(out=wot_t, in_=wot[:, :])
            cbbf_t = pp.tile([128, BB_W], BF16, tag="cbbf")
            nc.sync.dma_start(out=cbbf_t, in_=cbbf[:, :])

            apar_t = cb32_t[:, 0:4]
            cosP = cb32_t[:, 4:5]
            sinPs = cb32_t[:, 5:6]
            cbc_t = cb16_t[:, CB_CBC:CB_CBC + S]
            sbc_t = cb16_t[:, CB_SBC:CB_SBC + S]
            cbcb_t = cb16_t[:, CB_CBCB:CB_CBCB + NB]
            sbcb_t = cb16_t[:, CB_SBCB:CB_SBCB + NB]
            oblk_t = cb16_t[:, CB_OBLK:CB_OBLK + 144]
            permw_t = cb16_t[:, CB_PERM:CB_PERM + 128]
            ident_t = cb16_t[:, CB_IDENT:CB_IDENT + 128]
            ind_t = cb16_t[0:4, CB_IND:CB_IND + 256]
            maskb_t = cbbf_t[0:NBD, BB_MASK:BB_MASK + S]
            obv_t = cbbf_t[0:NBD, BB_OBV:BB_OBV + 8]
            indb_t = cbbf_t[0:4, BB_INDB:BB_INDB + 256]

            asig = pp.tile([128, 4], F32, tag="asig")
            nc.scalar.activation(asig, apar_t, AF.Sigmoid)
            oma = pp.tile([128, 4], F32, tag="oma")  # 1 - sigmoid(a)
            nc.vector.tensor_scalar(out=oma, in0=asig, scalar1=-1.0,
                                    scalar2=1.0, op0=AL.mult, op1=AL.add)

            # ============ SSM scans (k and v fused per d-tile) ============
            kpre = [pp.tile([128, S], F16, tag=f"kp{dt}", name=f"kpre{dt}")
                    for dt in range(2)]
            v_t = [pp.tile([128, S], F16, tag=f"v{dt}", name=f"v_t{dt}")
                   for dt in range(2)]
            for dt in range(2):
                A4 = sio.tile([128, 2 * S], F16, tag="sc8k")
                B4 = sio.tile([128, 2 * S], F16, tag="sc8k")
                nc.vector.tensor_scalar(out=A4[:, 0:S], in0=nrs_t,
                                        scalar1=asig[:, dt:dt + 1],
                                        scalar2=None, op0=AL.mult)
                nc.vector.tensor_scalar(out=A4[:, S:2 * S], in0=nrs_t,
                                        scalar1=asig[:, 2 + dt:3 + dt],
                                        scalar2=None, op0=AL.mult)
                nc.vector.tensor_scalar(out=B4[:, 0:S], in0=x_t[dt],
                                        scalar1=oma[:, dt:dt + 1],
                                        scalar2=None, op0=AL.mult)
                nc.vector.tensor_scalar(out=B4[:, S:2 * S], in0=x_t[dt],
                                        scalar1=oma[:, 2 + dt:3 + dt],
                                        scalar2=None, op0=AL.mult)
                kv = sio.tile([128, 2 * S], F16, tag="sc8k")
                nc.vector.tensor_tensor_scan(out=kv, data0=A4, data1=B4,
                                             initial=0.0, op0=AL.mult,
                                             op1=AL.add)
                nc.vector.tensor_add(out=kpre[dt], in0=kv[:, 0:S],
                                     in1=x_t[dt])
                nc.vector.tensor_add(out=v_t[dt], in0=kv[:, S:2 * S],
                                     in1=x_t[dt])

            # ============ kd: constant rotation of kpre (for sub-diag) ======
            # kd = cosP*k + sinPs*shuffle(k); shuffle via PE permutation.
            kd = [pp.tile([128, S], F16, tag=f"kd{dt}", name=f"kd{dt}")
                  for dt in range(2)]
            for dt in range(2):
                t2k = sio.tile([128, S], F16, tag="t2k", bufs=1)
                for c in range(NCHUNK):
                    chs = slice(c * CS, (c + 1) * CS)
                    zp = psB.tile([128, CS], F32, tag="psb")
                    nc.tensor.matmul(zp, permw_t, kpre[dt][:, chs],
                                     start=True, stop=True)
                    nc.scalar.activation(t2k[:, chs], zp, AF.Copy,
                                         scale=sinPs)
                t1k = sio.tile([128, S], F16, tag="t1k", bufs=1)
                nc.vector.tensor_scalar(out=t1k, in0=kpre[dt], scalar1=cosP,
                                        scalar2=None, op0=AL.mult)
                nc.vector.tensor_add(out=kd[dt], in0=t1k, in1=t2k)

            # ============ boundary keys (gather 33 cols, rope there) =======
            kbd = [pp.tile([128, NBD], F16, tag=f"kbd{dt}", name=f"kbd{dt}")
                   for dt in range(2)]
            vbT = [pp.tile([128, 64], BF16, tag=f"vbT{dt}", name=f"vbT{dt}")
                   for dt in range(2)]
            for dt in range(2):
                kb = ck.tile([128, NB], F16, tag="kb")
                vb = ck.tile([128, NB], F16, tag="vb")
                for src_t, dst_t in ((kpre[dt], kb), (v_t[dt], vb)):
                    nc.vector.memset(dst_t[:, 33:NB], 0.0)
                    nc.vector.tensor_copy(out=dst_t[:, 0:1], in_=src_t[:, 0:1])
                    nc.vector.tensor_copy(
                        out=dst_t[:, 1:32],
                        in_=src_t.rearrange("p (a b) -> p a b", b=64)[:, 0:31, 47])
                    nc.vector.tensor_copy(out=dst_t[:, 32:33],
                                          in_=src_t[:, S - 1:S])
                # rope kb at boundary positions
                zb = ck.tile([128, NB], F16, tag="zb")
                nc.vector.stream_shuffle(zb, kb, SHUF_XOR1)
                t1b = ck.tile([128, NB], F16, tag="t1b")
                nc.vector.tensor_mul(out=t1b, in0=kb, in1=cbcb_t)
                t2b = ck.tile([128, NB], F16, tag="t2b")
                nc.vector.tensor_mul(out=t2b, in0=zb, in1=sbcb_t)
                krb = ck.tile([128, NB], F16, tag="krb")
                nc.vector.tensor_add(out=krb, in0=t1b, in1=t2b)
                # blockdiag [128, NBD]
                nc.vector.memset(kbd[dt], 0.0)
                nc.vector.tensor_copy(out=kbd[dt][0:64, 0:48], in_=krb[0:64, :])
                nc.vector.tensor_copy(out=kbd[dt][64:128, 64:112],
                                      in_=krb[64:128, :])
                # vbT: transpose vb blocks -> [48, 64] per head-half
                for hh in range(2):
                    tp = psT.tile([128, CS // 2], F16, tag="psbh",
                                  name=f"tp{dt}_{hh}")
                    nc.tensor.transpose(tp[0:48, 0:64],
                                        vb[hh * 64:(hh + 1) * 64, 0:48],
                                        ident_t[hh * 64:(hh + 1) * 64,
                                                hh * 64:(hh + 1) * 64],
                                        tile_position=(hh * 64, 0))
                    nc.scalar.activation(vbT[dt][hh * 64:hh * 64 + 48, :],
                                         tp[0:48, 0:64], AF.Copy)

            # ============ main per-chunk pipeline ============
            for c in range(NCHUNK):
                cl = c * CS
                chs = slice(cl, cl + CS)
                # --- Q projection ---
                accs = [psQ.tile([128, CS], F32, tag="mmacc",
                                 name=f"qacc{c}_{m}") for m in range(2)]
                for m in range(2):
                    for k in range(8):
                        nc.tensor.matmul(accs[m],
                                         wqt_t[:, k * CH + m * 128:
                                               k * CH + (m + 1) * 128],
                                         x_t[k][:, chs],
                                         start=(k == 0), stop=(k == 7))
                xq = [ch.tile([128, CS], F16, tag="xq", name=f"xq{c}_{dt}")
                      for dt in range(2)]
                for dt in range(2):
                    nc.scalar.activation(xq[dt], accs[dt], AF.Copy)
                # --- q rope (perm via PE; muls DVE+Pool) ---
                qr = [ch.tile([128, CS], F16, tag="qr", name=f"qr{c}_{dt}")
                      for dt in range(2)]
                for dt in range(2):
                    zq = psB.tile([128, CS], F32, tag="psb")
                    nc.tensor.matmul(zq, permw_t, xq[dt], start=True, stop=True)
                    zqs = ch.tile([128, CS], F16, tag="zqs")
                    nc.scalar.activation(zqs, zq, AF.Copy)
                    tq1 = ch.tile([128, CS], F16, tag="tq1")
                    nc.vector.tensor_mul(out=tq1, in0=xq[dt],
                                         in1=cbc_t[:, chs])
                    tq2 = ch.tile([128, CS], F16, tag="tq2")
                    nc.gpsimd.tensor_mul(out=tq2, in0=zqs, in1=sbc_t[:, chs])
                    nc.vector.tensor_add(out=qr[dt], in0=tq1, in1=tq2)

                # --- band scores (dt-accumulated, s1 rows 0:4 / s0 rows 4:8) --
                sp = psB.tile([128, CS], F32, tag="psb")
                prods = []
                for dt in range(2):
                    p1t = ch.tile([128, CS], F16, tag="prod", bufs=4)
                    nc.vector.tensor_mul(out=p1t, in0=xq[dt],
                                         in1=kpre[dt][:, chs])
                    p0t = ch.tile([128, CS], F16, tag="prod", bufs=4)
                    if c == 0:
                        nc.vector.memset(p0t[:, 0:1], 0.0)
                        nc.vector.tensor_mul(out=p0t[:, 1:CS],
                                             in0=xq[dt][:, 1:CS],
                                             in1=kd[dt][:, 0:CS - 1])
                    else:
                        nc.vector.tensor_mul(out=p0t, in0=xq[dt],
                                             in1=kd[dt][:, cl - 1:cl + CS - 1])
                    prods.append((p1t, p0t))
                for i, (dt, pi) in enumerate(((0, 0), (0, 1),
                                              (1, 0), (1, 1))):
                    nc.tensor.matmul(sp[0:36, :],
                                     oblk_t[:, 36 * i:36 * i + 36],
                                     prods[dt][pi], start=(i == 0),
                                     stop=(i == 3))
                if c == 0:
                    nc.vector.memset(sp[32:36, 0:1], NEG)
                e1 = cs.tile([4, CS], BF16, tag="erow", bufs=8, name=f"e1_{c}")
                nc.scalar.activation(e1, sp[0:4, :], AF.Exp, scale=0.125)
                e0 = cs.tile([4, CS], BF16, tag="erow", bufs=8, name=f"e0_{c}")
                nc.scalar.activation(e0, sp[32:36, :], AF.Exp, scale=0.125)

                # --- boundary scores ---
                embd = [ch.tile([NBD, CS], BF16, tag="embd", bufs=6,
                                name=f"embd{c}_{dt}") for dt in range(2)]
                for dt in range(2):
                    sb = psB.tile([128, CS], F32, tag="psb")
                    nc.tensor.matmul(sb[0:NBD, :], kbd[dt], qr[dt],
                                     start=True, stop=True)
                    eb = ch.tile([NBD, CS], BF16, tag="ebx")
                    nc.scalar.activation(eb, sb[0:NBD, :], AF.Exp, scale=0.125)
                    nc.gpsimd.tensor_mul(out=embd[dt], in0=eb,
                                         in1=maskb_t[:, chs])

                # --- denominator / p rows ---
                dbp = psB.tile([128, CS], F32, tag="psb")
                for dt in range(2):
                    nc.tensor.matmul(dbp[0:4, :], obv_t[:, 4 * dt:4 * dt + 4],
                                     embd[dt], start=(dt == 0), stop=(dt == 1))
                den = cs.tile([4, CS], BF16, tag="erow", bufs=8)
                nc.vector.tensor_add(out=den, in0=e1, in1=e0)
                den2 = cs.tile([4, CS], BF16, tag="erow", bufs=8)
                nc.vector.tensor_add(out=den2, in0=den, in1=dbp[0:4, :])
                rd = cs.tile([4, CS], BF16, tag="erow", bufs=8)
                nc.vector.reciprocal(rd, den2)
                p1 = cs.tile([4, CS], F16, tag="prow", bufs=4)
                p0 = cs.tile([4, CS], F16, tag="prow", bufs=4)
                nc.gpsimd.tensor_mul(out=p1, in0=e1, in1=rd)
                nc.gpsimd.tensor_mul(out=p0, in0=e0, in1=rd)

                # --- broadcasts + PV + combine ---
                attn = [ch.tile([128, CS], F16, tag="attn",
                                name=f"attn{c}_{dt}") for dt in range(2)]
                for dt in range(2):
                    io = dt * 128
                    p1b = psB.tile([128, CS], F32, tag="psb")
                    nc.tensor.matmul(p1b, ind_t[:, io:io + 128], p1,
                                     start=True, stop=True)
                    p0b = psB.tile([128, CS], F32, tag="psb")
                    nc.tensor.matmul(p0b, ind_t[:, io:io + 128], p0,
                                     start=True, stop=True)
                    rdb = psB.tile([128, CS], F32, tag="psb")
                    nc.tensor.matmul(rdb, indb_t[:, io:io + 128], rd,
                                     start=True, stop=True)
                    pv = psB.tile([128, CS], F32, tag="psb")
                    for hh in range(2):
                        nc.tensor.matmul(
                            pv[hh * 64:(hh + 1) * 64, :],
                            vbT[dt][hh * 64:hh * 64 + 48, :],
                            embd[dt][hh * 64:hh * 64 + 48, :],
                            start=True, stop=True,
                            tile_position=(hh * 64, hh * 64))
                    # SBUF staging: p1b/p0b/pvs on ACT; m3 takes rdb from PSUM
                    p1s = ch.tile([128, CS], F16, tag="pbs", bufs=9)
                    nc.scalar.activation(p1s, p1b, AF.Copy)
                    p0s = ch.tile([128, CS], F16, tag="pbs", bufs=9)
                    nc.scalar.activation(p0s, p0b, AF.Copy)
                    pvs = ch.tile([128, CS], BF16, tag="pbs", bufs=9)
                    nc.scalar.activation(pvs, pv, AF.Copy)
                    m1 = ch.tile([128, CS], F16, tag="mt", bufs=12)
                    nc.vector.tensor_mul(out=m1, in0=v_t[dt][:, chs], in1=p1s)
                    m2 = ch.tile([128, CS], F16, tag="mt", bufs=12)
                    if c == 0:
                        nc.vector.memset(m2[:, 0:1], 0.0)
                        nc.vector.tensor_mul(out=m2[:, 1:CS],
                                             in0=v_t[dt][:, 0:CS - 1],
                                             in1=p0s[:, 1:CS])
                    else:
                        nc.vector.tensor_mul(out=m2,
                                             in0=v_t[dt][:, cl - 1:cl + CS - 1],
                                             in1=p0s)
                    m3 = ch.tile([128, CS], F16, tag="mt", bufs=12)
                    nc.vector.tensor_mul(out=m3, in0=pvs, in1=rdb)
                    m4 = ch.tile([128, CS], F16, tag="mt", bufs=12)
                    nc.vector.tensor_add(out=m4, in0=m1, in1=m2)
                    nc.vector.tensor_add(out=attn[dt], in0=m4, in1=m3)

                # --- output projection ---
                last = (c == NCHUNK - 1)
                for m in range(8):
                    use_b = last and m % 2 == 1
                    opool = psB if use_b else psO
                    acc = opool.tile([128, CS], F32,
                                     tag="psb" if use_b else "oacc",
                                     name=f"oacc{c}_{m}")
                    for k in range(2):
                        nc.tensor.matmul(acc,
                                         wot_t[:, k * D + m * 128:
                                               k * D + (m + 1) * 128],
                                         attn[k], start=(k == 0), stop=(k == 1))
                    stage = ch.tile([128, CS], F16, tag="ostage", bufs=8,
                                    name=f"ost{c}_{m}")
                    if m % 2 == 0:
                        nc.scalar.activation(stage, acc, AF.Copy)
                    else:
                        nc.vector.tensor_copy(out=stage, in_=acc)
                    nc.scalar.dma_start(out=outp[m * 128:(m + 1) * 128, chs],
                                        in_=stage)
    if cap_waits:
        _cap_sync_waits(nc)
    return nc


# ---------------- host side ----------------

def _host_consts(fc):
    C = np.zeros((128, S), np.float32)
    Sg = np.zeros((128, S), np.float32)
    for p in range(128):
        i = (p % 64) // 2
        if p % 2 == 0:
            C[p] = fc[:, i, 0, 0]
            Sg[p] = fc[:, i, 0, 1]
        else:
            C[p] = fc[:, i, 1, 1]
            Sg[p] = fc[:, i, 1, 0]
    starts = np.concatenate([[0], BND[:-1] + 1])
    nrs1 = np.ones(S, np.float32)
    nrs1[starts] = 0.0
    nrs = np.broadcast_to(nrs1, (128, S)).astype(np.float16).copy()
    # constant one-step rotation R(-delta): kd[2i] = cos k[2i] + sin k[2i+1]
    inv = (1.0 / (10000.0 ** (np.arange(0, HD, 2, dtype=np.float64) / HD)))
    rotc = np.zeros((128, 2), np.float32)
    for p in range(128):
        i = (p % 64) // 2
        rotc[p, 0] = np.cos(inv[i])
        rotc[p, 1] = np.sin(inv[i]) * (1.0 if p % 2 == 0 else -1.0)
    # fp16 const blob
    cb16 = np.zeros((128, CB16_W), np.float32)
    cb16[:, CB_CBC:CB_CBC + S] = C
    cb16[:, CB_SBC:CB_SBC + S] = Sg
    cb16[:, CB_CBCB + 0:CB_CBCB + NBR] = C[:, BND]
    cb16[:, CB_SBCB + 0:CB_SBCB + NBR] = Sg[:, BND]
    for i, (dt, pi) in enumerate(((0, 0), (0, 1), (1, 0), (1, 1))):
        base = CB_OBLK + 36 * i + 32 * pi + 2 * dt
        cb16[0:64, base + 0] = 1.0
        cb16[64:128, base + 1] = 1.0
    for p in range(128):
        cb16[p, CB_PERM + (p ^ 1)] = 1.0
        cb16[p, CB_IDENT + p] = 1.0
    ind = np.zeros((4, 256), np.float32)
    ind[0, 0:64] = 1.0
    ind[1, 64:128] = 1.0
    ind[2, 128:192] = 1.0
    ind[3, 192:256] = 1.0
    cb16[0:4, CB_IND:CB_IND + 256] = ind
    # bf16 blob
    cbbf = np.zeros((128, BB_W), np.float32)
    t = np.arange(S)
    for hh in range(2):
        for jb in range(NBR):
            cbbf[hh * 64 + jb, BB_MASK:BB_MASK + S] = (
                t >= BND[jb] + 2).astype(np.float32)
    cbbf[0:48, BB_OBV + 0] = 1.0
    cbbf[64:112, BB_OBV + 1] = 1.0
    cbbf[0:48, BB_OBV + 6] = 1.0
    cbbf[64:112, BB_OBV + 7] = 1.0
    cbbf[0:4, BB_INDB:BB_INDB + 256] = ind
    return nrs, rotc, cb16, cbbf


_prog = None


def make_in_maps(x, fc, wq_, wo_, a_k_, a_v_):
    nrs, rotc, cb16, cbbf = _host_consts(fc)
    import ml_dtypes
    bf = ml_dtypes.bfloat16
    cb16_h = cb16.astype(np.float16)
    cbbf_h = cbbf.astype(bf)
    in_maps, metas = [], []
    for b in range(B):
        xT = np.ascontiguousarray(x[b].T)
        for g in range(NG):
            c0 = g * CH
            perm = np.concatenate([np.arange(c0, c0 + CH),
                                   np.arange(0, c0),
                                   np.arange(c0 + CH, D)]).astype(np.int64)
            xt_core = np.ascontiguousarray(xT[perm]).astype(np.float16)
            wq_core = wq_[c0:c0 + CH, :].T[perm]          # [D, CH]
            wqt_pack = np.ascontiguousarray(
                wq_core.reshape(8, 128, CH).transpose(1, 0, 2).reshape(
                    128, 8 * CH)).astype(np.float16)
            wo_core = wo_[:, c0:c0 + CH].T                # [CH, D]
            wot_pack = np.ascontiguousarray(
                wo_core.reshape(2, 128, D).transpose(1, 0, 2).reshape(
                    128, 2 * D)).astype(np.float16)
            apar = np.stack([a_k_[c0:c0 + 128], a_k_[c0 + 128:c0 + 256],
                             a_v_[c0:c0 + 128], a_v_[c0 + 128:c0 + 256]],
                            axis=1).astype(np.float32)
            cb32_h = np.concatenate([apar, rotc], axis=1).astype(np.float32)
            in_maps.append({
                "xt": xt_core, "wqt": wqt_pack, "wot": wot_pack,
                "nrs": nrs, "cb16": cb16_h, "cb32": cb32_h, "cbbf": cbbf_h,
            })
            metas.append((b, g))
    return in_maps, metas


def kernel(x, freq_cis, wq, wo, a_k, a_v):
    global _prog
    x = np.asarray(x, np.float32)
    fc = np.asarray(freq_cis, np.float32)
    wq_ = np.asarray(wq, np.float32)
    wo_ = np.asarray(wo, np.float32)
    a_k_ = np.asarray(a_k, np.float32)
    a_v_ = np.asarray(a_v, np.float32)
    in_maps, metas = make_in_maps(x, fc, wq_, wo_, a_k_, a_v_)
    if _prog is None:
        _prog = build_program()
    res = run_bass_kernel_spmd(_prog, in_maps, core_ids=list(range(8)))
    out = np.zeros((B, S, D), np.float32)
    for (b, g), r in zip(metas, res.results):
        out[b] += np.asarray(r["outp"], np.float32).T
    return out


if __name__ == "__main__":
    build_program()
    print("program built ok")


# revision 7
# speedup vs baseline: 1.0234x; 1.0037x over previous
"""Trainium2 Bass kernel for AttentiveSSMNoProjCyc (sparse_attention), v2.

Sharding: 8 cores = 2 batches x 4 head-groups (4 heads / 256 channels each).
Per core, [channel, time] layout, fp16 compute domain / bf16 exp domain:
  - SSM scans via one fused tensor_tensor_scan [128, 2*S] per d-tile
  - band scores use rope cancellation: diag = q.k, sub-diag = q.(R(-d)k)
    with a constant per-partition rotation (no full k-rope)
  - boundary keys: gather 33 cols of k, rope them at [128,48] cost
  - q-rope via PE permutation matmul + ACT copy + DVE/Pool fp16 TT
  - per-chunk softmax / combine pipeline
  - DMAs consolidated into packed blobs (HWDGE generation is serialized)
Host sums the 4 per-batch partials and transposes back.
"""
import numpy as np

import concourse.bass as bass
import concourse.mybir as mybir
from concourse.bass_utils import run_bass_kernel_spmd
from concourse.tile import TileContext
import concourse.tile as _tile_mod
from concourse.vector_clock import ScopedClock as _ScopedClock


def _split_drain_and_barrier(self, tick_clock, wait_clock):
    """Tail drain, with its sem waits spread over chained SP nops."""
    probe = self.nc.sync.nop()
    wait_clock.add_sem_waits(
        probe.ins, _ScopedClock({None: tick_clock.global_clock})
    )
    si = probe.ins.sync_info
    waits = list(si.on_wait) if si is not None else []
    upds = list(si.on_update) if si is not None else []
    MAXW = 1
    if len(waits) > MAXW:
        probe.ins.sync_info = mybir.SyncInfo(on_wait=waits[:MAXW],
                                             on_update=upds)
        for i in range(MAXW, len(waits), MAXW):
            extra = self.nc.sync.nop()
            extra.ins.sync_info = mybir.SyncInfo(
                on_wait=waits[i:i + MAXW], on_update=[])
    self.nc.sync.drain()

    self.nc.all_engine_barrier()
    assert self.sems is not None
    popped = self.nc._tile_sem_poison_stack.pop()
    assert popped is self._sem_poison
    self.nc.clear_and_free_semaphores(list(self.sems.allocated().values()))
    self.nc.all_engine_barrier()


_tile_mod.TileContext._drain_and_barrier = _split_drain_and_barrier


def _cap_sync_waits(nc, cap=1):
    """Hoist excess sync waits onto same-engine carrier NOPs."""
    nid = [0]

    def mknop(engine, waits):
        nid[0] += 1
        nop = mybir.InstNoOp(name=f"I-capw-{nid[0]}", ins=[], outs=[])
        nop.engine = engine
        nop.sync_info = mybir.SyncInfo(on_wait=list(waits), on_update=[])
        return nop

    for bb in nc.m.functions[0].blocks:
        il = bb.instructions
        i = 0
        while i < len(il):
            ins = il[i]
            si = ins.sync_info
            nw = len(si.on_wait) if si is not None else 0
            if nw > cap:
                waits = list(si.on_wait)
                ins.sync_info = mybir.SyncInfo(on_wait=waits[:cap],
                                               on_update=list(si.on_update))
                rest = waits[cap:]
                pos = i
                for j in range(0, len(rest), cap):
                    il.insert(pos, mknop(ins.engine, rest[j:j + cap]))
                    pos += 1
                    i += 1
            i += 1


B, S, D, H, HD = 2, 2048, 1024, 16, 64
NG = 4            # head-groups per batch
CH = 256          # channels per core (4 heads)
NB = 48           # padded boundary columns (33 real)
NBD = 112         # blockdiag boundary cols: head0 -> 0:48, head1 -> 64:112
NCHUNK = 4
CS = S // NCHUNK  # 512
F32 = mybir.dt.float32
F16 = mybir.dt.float16
BF16 = mybir.dt.bfloat16
AL = mybir.AluOpType
AF = mybir.ActivationFunctionType
NEG = -1e30

SHUF_XOR1 = [i ^ 1 for i in range(32)]

# packed fp16 const blob layout (columns)
CB_CBC = 0                 # [128, S]
CB_SBC = CB_CBC + S        # [128, S]
CB_CBCB = CB_SBC + S       # [128, NB]
CB_SBCB = CB_CBCB + NB     # [128, NB]
CB_OBLK = CB_SBCB + NB     # [128, 144]: 4x 36-col variants
CB_PERM = CB_OBLK + 144    # [128, 128]
CB_IDENT = CB_PERM + 128   # [128, 128]
CB_IND = CB_IDENT + 128    # [4, 256]
CB16_W = CB_IND + 256

# packed bf16 blob layout
BB_MASK = 0                # [NBD, S]
BB_OBV = BB_MASK + S       # [NBD, 8]
BB_INDB = BB_OBV + 8       # [4, 256]
BB_W = BB_INDB + 256

# packed fp32 blob: apar [128,4] | rotc [128,2]
C32_W = 6


def _boundaries():
    K_, LAYER_, NLAYERS_, MAXLEN_ = 64, 4, 16, 16384
    off = min(K_ - 1, LAYER_ * (K_ // NLAYERS_))
    bl = [b - off for b in range(K_ - 1, MAXLEN_, K_)]
    if bl[-1] != MAXLEN_ - 1:
        bl.append(MAXLEN_ - 1)
    if bl[0] != 0:
        bl.insert(0, 0)
    b = np.asarray(bl)
    b = b[b < S].copy()
    b[-1] = S - 1
    return b


BND = _boundaries()
NBR = len(BND)  # 33


def build_program(cap_waits=True):
    nc = bass.Bass()
    dp = nc.declare_dram_parameter
    xt = dp("xt", [D, S], F16, isOutput=False)
    wqt = dp("wqt", [128, 8 * CH], F16, isOutput=False)   # packed k-tiles
    wot = dp("wot", [128, 2 * D], F16, isOutput=False)    # packed k-tiles
    nrs = dp("nrs", [128, S], F16, isOutput=False)
    cb16 = dp("cb16", [128, CB16_W], F16, isOutput=False)
    cb32 = dp("cb32", [128, C32_W], F32, isOutput=False)
    cbbf = dp("cbbf", [128, BB_W], BF16, isOutput=False)
    outp = dp("outp", [D, S], F16, isOutput=True)

    with TileContext(nc) as tc, nc.allow_low_precision(
            reason="2e-2 output tolerance; fp16/bf16 validated vs reference"):
        with (
            tc.tile_pool(name="persist", bufs=1) as pp,
            tc.tile_pool(name="scanio", bufs=3) as sio,     # A4/B4/kv 8K tiles
            tc.tile_pool(name="chk", bufs=3) as ch,         # per-chunk f16 tiles
            tc.tile_pool(name="chsm", bufs=3) as cs,        # per-chunk small rows
            tc.tile_pool(name="small", bufs=2) as ck,
            tc.tile_pool(name="psQ", bufs=2, space="PSUM") as psQ,
            tc.tile_pool(name="psB", bufs=4, space="PSUM") as psB,
            tc.tile_pool(name="psT", bufs=1, space="PSUM") as psT,
            tc.tile_pool(name="psO", bufs=2, space="PSUM") as psO,
        ):
            # ============ DMAs (ordered: scan-critical first) ============
            x_t = [pp.tile([128, S], F16, tag=f"x{k}", name=f"x_t{k}")
                   for k in range(8)]
            for k in range(2):
                nc.sync.dma_start(out=x_t[k], in_=xt[k * 128:(k + 1) * 128, :])
            nrs_t = pp.tile([128, S], F16, tag="nrs")
            nc.sync.dma_start(out=nrs_t, in_=nrs[:, :])
            cb32_t = pp.tile([128, C32_W], F32, tag="cb32")
            nc.sync.dma_start(out=cb32_t, in_=cb32[:, :])
            cb16_t = pp.tile([128, CB16_W], F16, tag="cb16")
            nc.sync.dma_start(out=cb16_t, in_=cb16[:, :])
            wqt_t = pp.tile([128, 8 * CH], F16, tag="wqt")
            nc.sync.dma_start(out=wqt_t, in_=wqt[:, :])
            for k in range(2, 8):
                nc.sync.dma_start(out=x_t[k], in_=xt[k * 128:(k + 1) * 128, :])
            wot_t = pp.tile([128, 2 * D], F16, tag="wot")
            nc.sync.dma_start(out=wot_t, in_=wot[:, :])
            cbbf_t = pp.tile([128, BB_W], BF16, tag="cbbf")
            nc.sync.dma_start(out=cbbf_t, in_=cbbf[:, :])

            apar_t = cb32_t[:, 0:4]
            cosP = cb32_t[:, 4:5]
            sinPs = cb32_t[:, 5:6]
            cbc_t = cb16_t[:, CB_CBC:CB_CBC + S]
            sbc_t = cb16_t[:, CB_SBC:CB_SBC + S]
            cbcb_t = cb16_t[:, CB_CBCB:CB_CBCB + NB]
            sbcb_t = cb16_t[:, CB_SBCB:CB_SBCB + NB]
            oblk_t = cb16_t[:, CB_OBLK:CB_OBLK + 144]
            permw_t = cb16_t[:, CB_PERM:CB_PERM + 128]
            ident_t = cb16_t[:, CB_IDENT:CB_IDENT + 128]
            ind_t = cb16_t[0:4, CB_IND:CB_IND + 256]
            maskb_t = cbbf_t[0:NBD, BB_MASK:BB_MASK + S]
            obv_t = cbbf_t[0:NBD, BB_OBV:BB_OBV + 8]
            indb_t = cbbf_t[0:4, BB_INDB:BB_INDB + 256]

            asig = pp.tile([128, 4], F32, tag="asig")
            nc.scalar.activation(asig, apar_t, AF.Sigmoid)
            oma = pp.tile([128, 4], F32, tag="oma")  # 1 - sigmoid(a)
            nc.vector.tensor_scalar(out=oma, in0=asig, scalar1=-1.0,
                                    scalar2=1.0, op0=AL.mult, op1=AL.add)

            # ============ SSM scans (k and v fused per d-tile) ============
            kpre = [pp.tile([128, S], F16, tag=f"kp{dt}", name=f"kpre{dt}")
                    for dt in range(2)]
            v_t = [pp.tile([128, S], F16, tag=f"v{dt}", name=f"v_t{dt}")
                   for dt in range(2)]
            for dt in range(2):
                A4 = sio.tile([128, 2 * S], F16, tag="sc8k")
                B4 = sio.tile([128, 2 * S], F16, tag="sc8k")
                nc.vector.tensor_scalar(out=A4[:, 0:S], in0=nrs_t,
                                        scalar1=asig[:, dt:dt + 1],
                                        scalar2=None, op0=AL.mult)
                nc.vector.tensor_scalar(out=A4[:, S:2 * S], in0=nrs_t,
                                        scalar1=asig[:, 2 + dt:3 + dt],
                                        scalar2=None, op0=AL.mult)
                nc.vector.tensor_scalar(out=B4[:, 0:S], in0=x_t[dt],
                                        scalar1=oma[:, dt:dt + 1],
                                        scalar2=None, op0=AL.mult)
                nc.vector.tensor_scalar(out=B4[:, S:2 * S], in0=x_t[dt],
                                        scalar1=oma[:, 2 + dt:3 + dt],
                                        scalar2=None, op0=AL.mult)
                kv = sio.tile([128, 2 * S], F16, tag="sc8k")
                nc.vector.tensor_tensor_scan(out=kv, data0=A4, data1=B4,
                                             initial=0.0, op0=AL.mult,
                                             op1=AL.add)
                nc.vector.tensor_add(out=kpre[dt], in0=kv[:, 0:S],
                                     in1=x_t[dt])
                nc.vector.tensor_add(out=v_t[dt], in0=kv[:, S:2 * S],
                                     in1=x_t[dt])

            # ============ kd: constant rotation of kpre (for sub-diag) ======
            # kd = cosP*k + sinPs*shuffle(k); shuffle via PE permutation.
            kd = [pp.tile([128, S], F16, tag=f"kd{dt}", name=f"kd{dt}")
                  for dt in range(2)]
            for dt in range(2):
                t2k = sio.tile([128, S], F16, tag="t2k", bufs=1)
                for c in range(NCHUNK):
                    chs = slice(c * CS, (c + 1) * CS)
                    zp = psB.tile([128, CS], F32, tag="psb")
                    nc.tensor.matmul(zp, permw_t, kpre[dt][:, chs],
                                     start=True, stop=True)
                    nc.scalar.activation(t2k[:, chs], zp, AF.Copy,
                                         scale=sinPs)
                t1k = sio.tile([128, S], F16, tag="t1k", bufs=1)
                nc.vector.tensor_scalar(out=t1k, in0=kpre[dt], scalar1=cosP,
                                        scalar2=None, op0=AL.mult)
                nc.vector.tensor_add(out=kd[dt], in0=t1k, in1=t2k)

            # ============ boundary keys (gather 33 cols, rope there) =======
            kbd = [pp.tile([128, NBD], F16, tag=f"kbd{dt}", name=f"kbd{dt}")
                   for dt in range(2)]
            vbT = [pp.tile([128, 64], BF16, tag=f"vbT{dt}", name=f"vbT{dt}")
                   for dt in range(2)]
            for dt in range(2):
                kb = ck.tile([128, NB], F16, tag="kb")
                vb = ck.tile([128, NB], F16, tag="vb")
                for src_t, dst_t in ((kpre[dt], kb), (v_t[dt], vb)):
                    nc.vector.memset(dst_t[:, 33:NB], 0.0)
                    nc.vector.tensor_copy(out=dst_t[:, 0:1], in_=src_t[:, 0:1])
                    nc.vector.tensor_copy(
                        out=dst_t[:, 1:32],
                        in_=src_t.rearrange("p (a b) -> p a b", b=64)[:, 0:31, 47])
                    nc.vector.tensor_copy(out=dst_t[:, 32:33],
                                          in_=src_t[:, S - 1:S])
                # rope kb at boundary positions
                zb = ck.tile([128, NB], F16, tag="zb")
                nc.vector.stream_shuffle(zb, kb, SHUF_XOR1)
                t1b = ck.tile([128, NB], F16, tag="t1b")
                nc.vector.tensor_mul(out=t1b, in0=kb, in1=cbcb_t)
                t2b = ck.tile([128, NB], F16, tag="t2b")
                nc.vector.tensor_mul(out=t2b, in0=zb, in1=sbcb_t)
                krb = ck.tile([128, NB], F16, tag="krb")
                nc.vector.tensor_add(out=krb, in0=t1b, in1=t2b)
                # blockdiag [128, NBD]
                nc.vector.memset(kbd[dt], 0.0)
                nc.vector.tensor_copy(out=kbd[dt][0:64, 0:48], in_=krb[0:64, :])
                nc.vector.tensor_copy(out=kbd[dt][64:128, 64:112],
                                      in_=krb[64:128, :])
                # vbT: transpose vb blocks -> [48, 64] per head-half
                for hh in range(2):
                    tp = psT.tile([128, CS // 2], F16, tag="psbh",
                                  name=f"tp{dt}_{hh}")
                    nc.tensor.transpose(tp[0:48, 0:64],
                                        vb[hh * 64:(hh + 1) * 64, 0:48],
                                        ident_t[hh * 64:(hh + 1) * 64,
                                                hh * 64:(hh + 1) * 64],
                                        tile_position=(hh * 64, 0))
                    nc.scalar.activation(vbT[dt][hh * 64:hh * 64 + 48, :],
                                         tp[0:48, 0:64], AF.Copy)

            # ============ main per-chunk pipeline ============
            for c in range(NCHUNK):
                cl = c * CS
                chs = slice(cl, cl + CS)
                # --- Q projection ---
                accs = [psQ.tile([128, CS], F32, tag="mmacc",
                                 name=f"qacc{c}_{m}") for m in range(2)]
                for m in range(2):
                    for k in range(8):
                        nc.tensor.matmul(accs[m],
                                         wqt_t[:, k * CH + m * 128:
                                               k * CH + (m + 1) * 128],
                                         x_t[k][:, chs],
                                         start=(k == 0), stop=(k == 7))
                xq = [ch.tile([128, CS], F16, tag="xq", name=f"xq{c}_{dt}")
                      for dt in range(2)]
                for dt in range(2):
                    nc.scalar.activation(xq[dt], accs[dt], AF.Copy)
                # --- q rope (perm via PE; muls DVE+Pool) ---
                qr = [ch.tile([128, CS], F16, tag="qr", name=f"qr{c}_{dt}")
                      for dt in range(2)]
                for dt in range(2):
                    zq = psB.tile([128, CS], F32, tag="psb")
                    nc.tensor.matmul(zq, permw_t, xq[dt], start=True, stop=True)
                    zqs = ch.tile([128, CS], F16, tag="zqs")
                    nc.scalar.activation(zqs, zq, AF.Copy)
                    tq1 = ch.tile([128, CS], F16, tag="tq1")
                    nc.vector.tensor_mul(out=tq1, in0=xq[dt],
                                         in1=cbc_t[:, chs])
                    tq2 = ch.tile([128, CS], F16, tag="tq2")
                    nc.vector.tensor_mul(out=tq2, in0=zqs, in1=sbc_t[:, chs])
                    nc.vector.tensor_add(out=qr[dt], in0=tq1, in1=tq2)

                # --- band scores (dt-accumulated, s1 rows 0:4 / s0 rows 4:8) --
                sp = psB.tile([128, CS], F32, tag="psb")
                prods = []
                for dt in range(2):
                    p1t = ch.tile([128, CS], F16, tag="prod", bufs=4)
                    nc.vector.tensor_mul(out=p1t, in0=xq[dt],
                                         in1=kpre[dt][:, chs])
                    p0t = ch.tile([128, CS], F16, tag="prod", bufs=4)
                    if c == 0:
                        nc.vector.memset(p0t[:, 0:1], 0.0)
                        nc.vector.tensor_mul(out=p0t[:, 1:CS],
                                             in0=xq[dt][:, 1:CS],
                                             in1=kd[dt][:, 0:CS - 1])
                    else:
                        nc.vector.tensor_mul(out=p0t, in0=xq[dt],
                                             in1=kd[dt][:, cl - 1:cl + CS - 1])
                    prods.append((p1t, p0t))
                for i, (dt, pi) in enumerate(((0, 0), (0, 1),
                                              (1, 0), (1, 1))):
                    nc.tensor.matmul(sp[0:36, :],
                                     oblk_t[:, 36 * i:36 * i + 36],
                                     prods[dt][pi], start=(i == 0),
                                     stop=(i == 3))
                if c == 0:
                    nc.vector.memset(sp[32:36, 0:1], NEG)
                e1 = cs.tile([4, CS], BF16, tag="erow", bufs=8, name=f"e1_{c}")
                nc.scalar.activation(e1, sp[0:4, :], AF.Exp, scale=0.125)
                e0 = cs.tile([4, CS], BF16, tag="erow", bufs=8, name=f"e0_{c}")
                nc.scalar.activation(e0, sp[32:36, :], AF.Exp, scale=0.125)

                # --- boundary scores ---
                embd = [ch.tile([NBD, CS], BF16, tag="embd", bufs=6,
                                name=f"embd{c}_{dt}") for dt in range(2)]
                for dt in range(2):
                    sb = psB.tile([128, CS], F32, tag="psb")
                    nc.tensor.matmul(sb[0:NBD, :], kbd[dt], qr[dt],
                                     start=True, stop=True)
                    eb = ch.tile([NBD, CS], BF16, tag="ebx")
                    nc.scalar.activation(eb, sb[0:NBD, :], AF.Exp, scale=0.125)
                    nc.gpsimd.tensor_mul(out=embd[dt], in0=eb,
                                         in1=maskb_t[:, chs])

                # --- denominator / p rows ---
                dbp = psB.tile([128, CS], F32, tag="psb")
                for dt in range(2):
                    nc.tensor.matmul(dbp[0:4, :], obv_t[:, 4 * dt:4 * dt + 4],
                                     embd[dt], start=(dt == 0), stop=(dt == 1))
                den = cs.tile([4, CS], BF16, tag="erow", bufs=8)
                nc.vector.tensor_add(out=den, in0=e1, in1=e0)
                den2 = cs.tile([4, CS], BF16, tag="erow", bufs=8)
                nc.vector.tensor_add(out=den2, in0=den, in1=dbp[0:4, :])
                rd = cs.tile([4, CS], BF16, tag="erow", bufs=8)
                nc.vector.reciprocal(rd, den2)
                p1 = cs.tile([4, CS], F16, tag="prow", bufs=4)
                p0 = cs.tile([4, CS], F16, tag="prow", bufs=4)
                nc.gpsimd.tensor_mul(out=p1, in0=e1, in1=rd)
                nc.gpsimd.tensor_mul(out=p0, in0=e0, in1=rd)

                # --- broadcasts + PV + combine ---
                attn = [ch.tile([128, CS], F16, tag="attn",
                                name=f"attn{c}_{dt}") for dt in range(2)]
                for dt in range(2):
                    io = dt * 128
                    p1b = psB.tile([128, CS], F32, tag="psb")
                    nc.tensor.matmul(p1b, ind_t[:, io:io + 128], p1,
                                     start=True, stop=True)
                    p0b = psB.tile([128, CS], F32, tag="psb")
                    nc.tensor.matmul(p0b, ind_t[:, io:io + 128], p0,
                                     start=True, stop=True)
                    rdb = psB.tile([128, CS], F32, tag="psb")
                    nc.tensor.matmul(rdb, indb_t[:, io:io + 128], rd,
                                     start=True, stop=True)
                    pv = psB.tile([128, CS], F32, tag="psb")
                    for hh in range(2):
                        nc.tensor.matmul(
                            pv[hh * 64:(hh + 1) * 64, :],
                            vbT[dt][hh * 64:hh * 64 + 48, :],
                            embd[dt][hh * 64:hh * 64 + 48, :],
                            start=True, stop=True,
                            tile_position=(hh * 64, hh * 64))
                    # SBUF staging: p1b/p0b/pvs on ACT; m3 takes rdb from PSUM
                    p1s = ch.tile([128, CS], F16, tag="pbs", bufs=9)
                    nc.scalar.activation(p1s, p1b, AF.Copy)
                    p0s = ch.tile([128, CS], F16, tag="pbs", bufs=9)
                    nc.scalar.activation(p0s, p0b, AF.Copy)
                    pvs = ch.tile([128, CS], BF16, tag="pbs", bufs=9)
                    nc.scalar.activation(pvs, pv, AF.Copy)
                    m1 = ch.tile([128, CS], F16, tag="mt", bufs=12)
                    nc.vector.tensor_mul(out=m1, in0=v_t[dt][:, chs], in1=p1s)
                    m2 = ch.tile([128, CS], F16, tag="mt", bufs=12)
                    if c == 0:
                        nc.vector.memset(m2[:, 0:1], 0.0)
                        nc.gpsimd.tensor_mul(out=m2[:, 1:CS],
                                             in0=v_t[dt][:, 0:CS - 1],
                                             in1=p0s[:, 1:CS])
                    else:
                        nc.gpsimd.tensor_mul(out=m2,
                                             in0=v_t[dt][:, cl - 1:cl + CS - 1],
                                             in1=p0s)
                    m3 = ch.tile([128, CS], F16, tag="mt", bufs=12)
                    nc.vector.tensor_mul(out=m3, in0=pvs, in1=rdb)
                    m4 = ch.tile([128, CS], F16, tag="mt", bufs=12)
                    nc.vector.tensor_add(out=m4, in0=m1, in1=m2)
                    nc.vector.tensor_add(out=attn[dt], in0=m4, in1=m3)

                # --- output projection ---
                last = (c == NCHUNK - 1)
                for m in range(8):
                    use_b = last and m % 2 == 1
                    opool = psB if use_b else psO
                    acc = opool.tile([128, CS], F32,
                                     tag="psb" if use_b else "oacc",
                                     name=f"oacc{c}_{m}")
                    for k in range(2):
                        nc.tensor.matmul(acc,
                                         wot_t[:, k * D + m * 128:
                                               k * D + (m + 1) * 128],
                                         attn[k], start=(k == 0), stop=(k == 1))
                    stage = ch.tile([128, CS], F16, tag="ostage", bufs=8,
                                    name=f"ost{c}_{m}")
                    if m % 2 == 0:
                        nc.scalar.activation(stage, acc, AF.Copy)
                    else:
                        nc.vector.tensor_copy(out=stage, in_=acc)
                    nc.scalar.dma_start(out=outp[m * 128:(m + 1) * 128, chs],
                                        in_=stage)
    if cap_waits:
        _cap_sync_waits(nc)
    return nc


# ---------------- host side ----------------

def _host_consts(fc):
    C = np.zeros((128, S), np.float32)
    Sg = np.zeros((128, S), np.float32)
    for p in range(128):
        i = (p % 64) // 2
        if p % 2 == 0:
            C[p] = fc[:, i, 0, 0]
            Sg[p] = fc[:, i, 0, 1]
        else:
            C[p] = fc[:, i, 1, 1]
            Sg[p] = fc[:, i, 1, 0]
    starts = np.concatenate([[0], BND[:-1] + 1])
    nrs1 = np.ones(S, np.float32)
    nrs1[starts] = 0.0
    nrs = np.broadcast_to(nrs1, (128, S)).astype(np.float16).copy()
    # constant one-step rotation R(-delta): kd[2i] = cos k[2i] + sin k[2i+1]
    inv = (1.0 / (10000.0 ** (np.arange(0, HD, 2, dtype=np.float64) / HD)))
    rotc = np.zeros((128, 2), np.float32)
    for p in range(128):
        i = (p % 64) // 2
        rotc[p, 0] = np.cos(inv[i])
        rotc[p, 1] = np.sin(inv[i]) * (1.0 if p % 2 == 0 else -1.0)
    # fp16 const blob
    cb16 = np.zeros((128, CB16_W), np.float32)
    cb16[:, CB_CBC:CB_CBC + S] = C
    cb16[:, CB_SBC:CB_SBC + S] = Sg
    cb16[:, CB_CBCB + 0:CB_CBCB + NBR] = C[:, BND]
    cb16[:, CB_SBCB + 0:CB_SBCB + NBR] = Sg[:, BND]
    for i, (dt, pi) in enumerate(((0, 0), (0, 1), (1, 0), (1, 1))):
        base = CB_OBLK + 36 * i + 32 * pi + 2 * dt
        cb16[0:64, base + 0] = 1.0
        cb16[64:128, base + 1] = 1.0
    for p in range(128):
        cb16[p, CB_PERM + (p ^ 1)] = 1.0
        cb16[p, CB_IDENT + p] = 1.0
    ind = np.zeros((4, 256), np.float32)
    ind[0, 0:64] = 1.0
    ind[1, 64:128] = 1.0
    ind[2, 128:192] = 1.0
    ind[3, 192:256] = 1.0
    cb16[0:4, CB_IND:CB_IND + 256] = ind
    # bf16 blob
    cbbf = np.zeros((128, BB_W), np.float32)
    t = np.arange(S)
    for hh in range(2):
        for jb in range(NBR):
            cbbf[hh * 64 + jb, BB_MASK:BB_MASK + S] = (
                t >= BND[jb] + 2).astype(np.float32)
    cbbf[0:48, BB_OBV + 0] = 1.0
    cbbf[64:112, BB_OBV + 1] = 1.0
    cbbf[0:48, BB_OBV + 6] = 1.0
    cbbf[64:112, BB_OBV + 7] = 1.0
    cbbf[0:4, BB_INDB:BB_INDB + 256] = ind
    return nrs, rotc, cb16, cbbf


_prog = None


def make_in_maps(x, fc, wq_, wo_, a_k_, a_v_):
    nrs, rotc, cb16, cbbf = _host_consts(fc)
    import ml_dtypes
    bf = ml_dtypes.bfloat16
    cb16_h = cb16.astype(np.float16)
    cbbf_h = cbbf.astype(bf)
    in_maps, metas = [], []
    for b in range(B):
        xT = np.ascontiguousarray(x[b].T)
        for g in range(NG):
            c0 = g * CH
            perm = np.concatenate([np.arange(c0, c0 + CH),
                                   np.arange(0, c0),
                                   np.arange(c0 + CH, D)]).astype(np.int64)
            xt_core = np.ascontiguousarray(xT[perm]).astype(np.float16)
            wq_core = wq_[c0:c0 + CH, :].T[perm]          # [D, CH]
            wqt_pack = np.ascontiguousarray(
                wq_core.reshape(8, 128, CH).transpose(1, 0, 2).reshape(
                    128, 8 * CH)).astype(np.float16)
            wo_core = wo_[:, c0:c0 + CH].T                # [CH, D]
            wot_pack = np.ascontiguousarray(
                wo_core.reshape(2, 128, D).transpose(1, 0, 2).reshape(
                    128, 2 * D)).astype(np.float16)
            apar = np.stack([a_k_[c0:c0 + 128], a_k_[c0 + 128:c0 + 256],
                             a_v_[c0:c0 + 128], a_v_[c0 + 128:c0 + 256]],
                            axis=1).astype(np.float32)
            cb32_h = np.concatenate([apar, rotc], axis=1).astype(np.float32)
            in_maps.append({
                "xt": xt_core, "wqt": wqt_pack, "wot": wot_pack,
                "nrs": nrs, "cb16": cb16_h, "cb32": cb32_h, "cbbf": cbbf_h,
            })
            metas.append((b, g))
    return in_maps, metas


def kernel(x, freq_cis, wq, wo, a_k, a_v):
    global _prog
    x = np.asarray(x, np.float32)
    fc = np.asarray(freq_cis, np.float32)
    wq_ = np.asarray(wq, np.float32)
    wo_ = np.asarray(wo, np.float32)
    a_k_ = np.asarray(a_k, np.float32)
    a_v_ = np.asarray(a_v, np.float32)
    in_maps, metas = make_in_maps(x, fc, wq_, wo_, a_k_, a_v_)
    if _prog is None:
        _prog = build_program()
    res = run_bass_kernel_spmd(_prog, in_maps, core_ids=list(range(8)))
    out = np.zeros((B, S, D), np.float32)
    for (b, g), r in zip(metas, res.results):
        out[b] += np.asarray(r["outp"], np.float32).T
    return out


if __name__ == "__main__":
    build_program()
    print("program built ok")
